# revision 1
# baseline (speedup 1.0000x reference)
"""AttentionPooling (ragged graph cross-attention pooling) on 8 TRN2 NeuronCores.

Strategy (SPMD, no collectives):
  * Host assigns 8 whole graphs to each of the 8 cores (serpentine by size),
    sorts each core's graphs by size into 8 "slots".  Slot j has a fixed tile
    count T[j] (shared by all cores, since the instruction stream is shared);
    each graph's edges are placed at its slot offset and zero-padded.
  * Host ships x^T (transposed edge features, bf16) per core + replicated
    weights.  Padding edges give exp(0)=1 in the softmax denominator, which is
    corrected with a host-computed per-slot pad count.
  * Softmax is computed without max-subtraction (scores ~ N(0,1); exp cannot
    overflow fp32) — mathematically identical to the reference's stable form.
  * Scores are linear in x: scores = (x @ w_k) . q  =  x @ Ws where
    Ws[:, (h,s)] = sum_d w_k[:, (h,d)] q[s,h,d] / sqrt(hd).  Ws ([256, 256])
    is host-precomputed from the weights and shipped fused with w_v as one
    [256, 512] operand, so the per-tile device work is:
      [v | sc][e, :] = x @ [w_v | Ws]    (PE, 2 matmuls/tile, N=512)
      ex             = exp(sc)           (ACT, psum->sbuf bf16)
      pooled[(h,s),(h,d)|denom] += ex.T @ [v | 1]  (PE, psum-accum per graph)
  * Per graph: denom -= npad; normalize by 1/denom (DVE); 32x32 block
    transpose (DVE StreamTranspose) to build the [128, (s,half)*8graphs]
    operand P2 for the MLP (w1 needs no permutation in this layout).
  * MLP: h1 = silu(pooled @ w1 + b1) (PE, 4-way tile_position-packed, +ACT),
    out = h1 @ w2 + b2 (PE), emitted as out^T [256, 8] per core; the host
    scatters core outputs into the final [64, 256].
"""

import os
import sys
from contextlib import ExitStack

import numpy as np

for _p in ("/opt/trn_rl_repo",):
    if _p not in sys.path:
        sys.path.append(_p)

import ml_dtypes  # noqa: E402

import concourse.bass as bass  # noqa: E402
import concourse.tile as tile  # noqa: E402
from concourse import mybir  # noqa: E402
from concourse.bass_utils import run_bass_kernel_spmd  # noqa: E402
from concourse.vector_clock import ScopedClock  # noqa: E402

BF16 = ml_dtypes.bfloat16

E, B, H, S, NH, HD = 131072, 64, 256, 32, 8, 32
NCORES = 8
NG = B // NCORES        # graphs (slots) per core
TILE = 128              # edge tile
GROUP = 512             # x^T DMA chunk (4 tiles)
SCALE = 1.0 / float(np.sqrt(HD))

AF = mybir.ActivationFunctionType

# ---------------------------------------------------------------------------
# Walrus workaround: this toolchain's InstDrain accepts only ONE sync wait;
# Tile's kernel-tail drain carries one wait per outstanding semaphore.
# Split it into a chain of single-wait drains.
_MAXW = 1


def _split_drain_and_barrier(self, tick_clock, wait_clock):
    nc = self.nc
    drain_inst = nc.sync.drain()
    wait_clock.add_sem_waits(
        drain_inst.ins, ScopedClock({None: tick_clock.global_clock})
    )
    waits = list(drain_inst.ins.sync_info.on_wait)
    if len(waits) > _MAXW:
        drain_inst.ins.sync_info = mybir.SyncInfo(on_wait=waits[:_MAXW], on_update=[])
        for i in range(_MAXW, len(waits), _MAXW):
            d2 = nc.sync.drain()
            d2.ins.sync_info = mybir.SyncInfo(
                on_wait=waits[i : i + _MAXW], on_update=[]
            )
    nc.all_engine_barrier()
    popped = nc._tile_sem_poison_stack.pop()
    assert popped is self._sem_poison
    nc.clear_and_free_semaphores(list(self.sems.allocated().values()))
    nc.all_engine_barrier()


tile.TileContext._drain_and_barrier = _split_drain_and_barrier

# Engine instructions are capped at 2 sync waits by this walrus (Drain/NoOp
# at 1).  Tile's sem-assignment occasionally emits more.  Hoist the excess
# onto single-wait NoOps inserted just before, on the same engine — the
# engine stalls at the NoOp instead, which is semantically identical.
_WAIT_CAP = {"InstDrain": 1}
_WAIT_CAP_DEFAULT = 1


def _fix_excess_waits(nc):
    n_fixed = 0
    for fn in nc.m.functions:
        for bb in fn.blocks:
            insts = bb.instructions
            out = []
            changed = False
            for inst in insts:
                si = inst.sync_info
                waits = list(si.on_wait) if si is not None else []
                cap = _WAIT_CAP.get(type(inst).__name__, _WAIT_CAP_DEFAULT)
                if len(waits) > cap:
                    changed = True
                    n_fixed += 1
                    excess = waits[: len(waits) - cap]
                    for i, w in enumerate(excess):
                        nop = mybir.InstNoOp(
                            name=f"{inst.name}-hw{i}", ins=[], outs=[]
                        )
                        nop.engine = inst.engine
                        nop.sync_info = mybir.SyncInfo(on_wait=[w], on_update=[])
                        out.append(nop)
                    inst.sync_info = mybir.SyncInfo(
                        on_wait=waits[len(excess) :], on_update=list(si.on_update)
                    )
                out.append(inst)
            if changed:
                bb.instructions = out
    return n_fixed

# ---------------------------------------------------------------------------

_PROGRAM_CACHE: dict[tuple, "bass.Bass"] = {}
LAST_RESULTS = None  # BassKernelResults of the most recent run (for testing)


def _install_ntff_hook_shim():
    """The image's antenv lacks axon_hooks; recreate it so trace=True works."""
    try:
        import types

        import antenv

        if "antenv.axon_hooks" not in sys.modules:
            mod = types.ModuleType("antenv.axon_hooks")
            mod._hook = None

            def set_axon_ntff_profile_hook(h):
                mod._hook = h

            def get_axon_ntff_profile_hook():
                return mod._hook

            mod.set_axon_ntff_profile_hook = set_axon_ntff_profile_hook
            mod.get_axon_ntff_profile_hook = get_axon_ntff_profile_hook
            sys.modules["antenv.axon_hooks"] = mod
            antenv.axon_hooks = mod
        import antenv.axon_hooks as ah

        if ah.get_axon_ntff_profile_hook() is None:
            from trn_agent_boot.trn_boot import _ntff_profile_via_ctypes

            ah.set_axon_ntff_profile_hook(
                _ntff_profile_via_ctypes("/opt/axon/libaxon_pjrt.so")
            )
    except Exception:
        pass


_install_ntff_hook_shim()

# Optional experiment: let walrus double-buffer LDWEIGHTS (default off here).
import concourse.bass_utils as _bass_utils  # noqa: E402

_orig_run_command = _bass_utils.run_command


def _run_command_ldwopt(cmd, **kw):
    if isinstance(cmd, list):
        cmd = [
            "--enable-ldw-opt=true" if c == "--enable-ldw-opt=false" else c
            for c in cmd
        ]
    return _orig_run_command(cmd, **kw)


if os.environ.get("KERNEL_LDW_OPT") == "1":
    _bass_utils.run_command = _run_command_ldwopt


def build_program(slot_tiles: tuple[int, ...]) -> "bass.Bass":
    """Build the SPMD Bass program for per-core slot tile counts."""
    TT = sum(slot_tiles)
    EC = TT * TILE
    assert TT % (GROUP // TILE) == 0
    NGRP = TT // (GROUP // TILE)

    # per-tile slot id / first / last flags
    slot_of, first_of, last_of = [], [], []
    for j, tj in enumerate(slot_tiles):
        for t in range(tj):
            slot_of.append(j)
            first_of.append(t == 0)
            last_of.append(t == tj - 1)

    f32, bf16 = mybir.dt.float32, mybir.dt.bfloat16
    nc = bass.Bass("TRN2", target_bir_lowering=False, debug=False, num_devices=NCORES)

    xt_d = nc.dram_tensor("xt", [H, EC], bf16, kind="ExternalInput").ap()
    wvs_d = nc.dram_tensor("wvs", [H, 2 * H], bf16, kind="ExternalInput").ap()
    w1_d = nc.dram_tensor("w1", [S * H, H], bf16, kind="ExternalInput").ap()
    w2_d = nc.dram_tensor("w2", [H, H], bf16, kind="ExternalInput").ap()
    b1_d = nc.dram_tensor("b1", [NG, H], f32, kind="ExternalInput").ap()
    b2_d = nc.dram_tensor("b2", [H, 1], f32, kind="ExternalInput").ap()
    npad_d = nc.dram_tensor("npad", [128, NG], f32, kind="ExternalInput").ap()
    ident_d = nc.dram_tensor("ident", [128, 128], bf16, kind="ExternalInput").ap()
    qsel_d = nc.dram_tensor("qsel", [128, NG], bf16, kind="ExternalInput").ap()
    outT_d = nc.dram_tensor("outT", [H, NG], f32, kind="ExternalOutput").ap()

    with tile.TileContext(nc) as tc, ExitStack() as ctx:
        const = ctx.enter_context(tc.tile_pool(name="const", bufs=1))
        w2_sb = const.tile([128, 2 * H], bf16)
        wvs_sb = const.tile([128, 2 * 2 * H], bf16)  # k-tile k: [wv_k | ws_k]
        w1_sb = const.tile([128, 64 * H], bf16)
        ident_sb = const.tile([128, 128], bf16)
        qsel_sb = const.tile([128, NG], bf16)
        b1_sb = const.tile([NG, H], f32)
        b2_sb = const.tile([128, 2], f32)
        npad_sb = const.tile([128, NG], f32)
        P2 = const.tile([128, 64 * NG], bf16)

        for k in range(2):
            r = slice(k * 128, (k + 1) * 128)
            nc.scalar.dma_start(wvs_sb[:, k * 2 * H : (k + 1) * 2 * H], wvs_d[r, :])
        nc.scalar.dma_start(npad_sb[:], npad_d[:])
        for k in range(2):
            r = slice(k * 128, (k + 1) * 128)
            nc.scalar.dma_start(w2_sb[:, k * H : (k + 1) * H], w2_d[r, :])
            nc.scalar.dma_start(b2_sb[:, k : k + 1], b2_d[r, :])
        nc.scalar.dma_start(ident_sb[:], ident_d[:])
        nc.scalar.dma_start(qsel_sb[:], qsel_d[:])
        nc.scalar.dma_start(b1_sb[:], b1_d[:])

        # Warm the ACT function tables while the first DMAs are in flight,
        # so the table loads are off the critical path.
        warm = const.tile([1, 2], f32)
        nc.gpsimd.memset(warm[:, 0:1], 0.0)
        nc.scalar.activation(warm[:, 1:2], warm[:, 0:1], AF.Exp)
        nc.scalar.activation(warm[:, 1:2], warm[:, 0:1], AF.Sigmoid)

        # ---- main edge loop ---------------------------------------------
        xt_pool = ctx.enter_context(tc.tile_pool(name="xtp", bufs=4))
        ex_pool = ctx.enter_context(tc.tile_pool(name="exp", bufs=6))
        ext_pool = ctx.enter_context(tc.tile_pool(name="ext", bufs=2))

        NRING = 6
        vs_ring = [const.tile([128, 258], bf16, name=f"vsring{i}") for i in range(NRING)]
        for t in vs_ring:
            nc.vector.memset(t[:, 128:129], 1.0)
            nc.vector.memset(t[:, 257:258], 1.0)

        pooled_tiles: list = [None, None]

        def emit_pooled(sl, fi, la, ex, vs):
            if fi:
                pooled_tiles[0] = pl_pool.tile([128, 129], f32, tag="pl0", name=f"pl0_s{sl}")
                pooled_tiles[1] = pl_pool.tile([128, 129], f32, tag="pl1", name=f"pl1_s{sl}")
            for m in range(2):
                nc.tensor.matmul(
                    pooled_tiles[m][:],
                    ex[:, m * 128 : (m + 1) * 128],
                    vs[:, m * 129 : m * 129 + 129],
                    start=fi,
                    stop=la,
                )
            if la:
                extract_graph(sl, pooled_tiles)

        P2v = P2[:].rearrange("p (s x) -> p s x", x=2 * NG)

        def extract_graph(g, ptiles):
            copy_eng = nc.vector if g == NG - 1 else nc.gpsimd
            for m in range(2):
                den = ext_pool.tile([128, 1], f32, tag="den", name=f"den{g}_{m}")
                nc.vector.tensor_scalar_sub(
                    den[:], ptiles[m][:, 128:129], npad_sb[:, g : g + 1]
                )
                rec = ext_pool.tile([128, 1], f32, tag="rec", name=f"rec{g}_{m}")
                nc.vector.reciprocal(rec[:], den[:])
                pn = ext_pool.tile([128, 128], f32, tag="pn", name=f"pn{g}_{m}")
                nc.vector.tensor_scalar_mul(pn[:], ptiles[m][:, 0:128], rec[:])
                pt = ext_pool.tile([128, 128], f32, tag="pt", name=f"pt{g}_{m}")
                nc.vector.transpose(pt[:], pn[:])
                for hh in range(4):
                    rr = slice(hh * 32, (hh + 1) * 32)
                    src = pt[rr, hh * 32 : (hh + 1) * 32].rearrange(
                        "p (a o) -> p a o", o=1
                    )
                    copy_eng.tensor_copy(P2v[rr, :, m * NG + g : m * NG + g + 1], src)

        with (
            tc.tile_pool(name="vscp", bufs=3, space="PSUM") as vsc_pool,
            tc.tile_pool(name="plp", bufs=2, space="PSUM") as pl_pool,
        ):
            from collections import deque

            pending = deque()
            tidx = 0
            for grp in range(NGRP):
                xt = [
                    xt_pool.tile([128, GROUP], bf16, tag="xt", name=f"xt_{grp}_{i}")
                    for i in range(2)
                ]
                for k in range(2):
                    nc.sync.dma_start(
                        xt[k][:],
                        xt_d[k * 128 : (k + 1) * 128, grp * GROUP : (grp + 1) * GROUP],
                    )
                for sub in range(4):
                    sl, fi, la = slot_of[tidx], first_of[tidx], last_of[tidx]
                    e0 = sub * TILE
                    vsc = vsc_pool.tile([128, 512], f32, tag="vsc", name=f"vsc{tidx}")
                    for k in range(2):
                        nc.tensor.matmul(
                            vsc[:],
                            xt[k][:, e0 : e0 + TILE],
                            wvs_sb[:, k * 2 * H : (k + 1) * 2 * H],
                            start=(k == 0),
                            stop=(k == 1),
                        )
                    ex = ex_pool.tile([128, 256], bf16, tag="ex", name=f"ex{tidx}")
                    nc.scalar.activation(ex[:], vsc[:, H : 2 * H], AF.Exp)
                    vs = vs_ring[tidx % NRING]
                    nc.vector.tensor_copy(
                        vs[:].rearrange("p (b c) -> p b c", c=129)[:, :, 0:128],
                        vsc[:, 0:H].rearrange("p (b c) -> p b c", c=128),
                    )
                    pending.append((sl, fi, la, ex, vs))
                    while len(pending) > 2:
                        emit_pooled(*pending.popleft())
                    tidx += 1
            while pending:
                emit_pooled(*pending.popleft())

        # w1 load — one big blocked DMA on the Scalar HWDGE ring, so its 4MB
        # transfer cannot queue ahead of the edge-loop xt groups on the Sync
        # ring (the scheduler hoists it regardless of trace position).
        nc.scalar.dma_start(
            w1_sb[:].rearrange("p (k c) -> p k c", c=H),
            w1_d[:].rearrange("(k p) c -> p k c", p=128),
        )

        # ---- MLP tail ----------------------------------------------------
        with (
            tc.tile_pool(name="mlpp", bufs=2, space="PSUM") as mp,
            tc.tile_pool(name="mlps", bufs=2) as ms,
        ):
            h1pp = mp.tile([128, H], f32, tag="h1pp")
            for j in range(64):
                q = j % 4
                nc.tensor.matmul(
                    h1pp[q * 32 : q * 32 + NG, :],
                    P2[:, j * NG : (j + 1) * NG],
                    w1_sb[:, j * H : (j + 1) * H],
                    start=(j < 4),
                    stop=(j >= 60),
                    tile_position=(0, q * 32),
                    skip_group_check=True,
                )
            h1ps = ms.tile([128, H], bf16, tag="h1ps")
            nc.gpsimd.memset(h1ps[:], 0.0)
            for q in range(4):
                eng = nc.vector if q % 2 == 0 else nc.scalar
                if eng is nc.vector:
                    nc.vector.tensor_copy(
                        h1ps[q * 32 : q * 32 + NG, :], h1pp[q * 32 : q * 32 + NG, :]
                    )
                else:
                    nc.scalar.activation(
                        h1ps[q * 32 : q * 32 + NG, :],
                        h1pp[q * 32 : q * 32 + NG, :],
                        AF.Copy,
                    )
            h1p = mp.tile([NG, H], f32, tag="h1p")
            nc.tensor.matmul(h1p[:], qsel_sb[:], h1ps[:], start=True, stop=True)
            h1s = ms.tile([NG, H], f32, tag="h1s")
            nc.vector.tensor_add(h1s[:], h1p[:], b1_sb[:])
            h1g = ms.tile([NG, H], f32, tag="h1g")
            nc.scalar.activation(h1g[:], h1s[:], AF.Sigmoid)
            h1b = ms.tile([NG, H], bf16, tag="h1b")
            nc.vector.tensor_mul(h1b[:], h1s[:], h1g[:])
            h1t = []
            for m in range(2):
                h1tp = mp.tile([128, NG], bf16, tag="h1tp", name=f"h1tp{m}")
                nc.tensor.transpose(
                    h1tp[:], h1b[:, m * 128 : (m + 1) * 128], ident_sb[0:NG, 0:NG]
                )
                ht = ms.tile([128, NG], bf16, tag=f"h1t{m}")
                nc.vector.tensor_copy(ht[:], h1tp[:])
                h1t.append(ht)
            for m in range(2):
                otp = mp.tile([128, NG], f32, tag="otp", name=f"otp{m}")
                for k in range(2):
                    nc.tensor.matmul(
                        otp[:],
                        w2_sb[:, k * H + m * 128 : k * H + m * 128 + 128],
                        h1t[k][:],
                        start=(k == 0),
                        stop=(k == 1),
                    )
                osb = ms.tile([128, NG], f32, tag="osb", name=f"osb{m}")
                nc.vector.tensor_scalar_add(osb[:], otp[:], b2_sb[:, m : m + 1])
                nc.sync.dma_start(outT_d[m * 128 : (m + 1) * 128, :], osb[:])

    return nc


def get_program(slot_tiles: tuple[int, ...]) -> "bass.Bass":
    if slot_tiles not in _PROGRAM_CACHE:
        nc = build_program(slot_tiles)
        # HW-path only (CoreSim snapshots the program before this pass)
        _fix_excess_waits(nc)
        _PROGRAM_CACHE[slot_tiles] = nc
    return _PROGRAM_CACHE[slot_tiles]


# ---------------------------------------------------------------------------
# Host-side sharding / padding


def plan_shards(batch: np.ndarray):
    """Returns (assign [NCORES][NG] graph ids, slot_tiles tuple, sizes)."""
    sizes = np.bincount(batch, minlength=B).astype(np.int64)
    order = np.argsort(-sizes, kind="stable")
    assign = [[] for _ in range(NCORES)]
    for r in range(NG):
        row = order[r * NCORES : (r + 1) * NCORES]
        if r % 2 == 1:
            row = row[::-1]
        for c in range(NCORES):
            assign[c].append(int(row[c]))
    for c in range(NCORES):
        assign[c].sort(key=lambda g: -sizes[g])
    slot_tiles = []
    for j in range(NG):
        mx = max(sizes[assign[c][j]] for c in range(NCORES))
        slot_tiles.append(int(max(1, -(-mx // TILE))))
    # round total tiles up to a GROUP multiple (pad goes to the last slot)
    rem = (-sum(slot_tiles)) % (GROUP // TILE)
    slot_tiles[-1] += rem
    return assign, tuple(slot_tiles), sizes


def make_in_maps(edge_features, batch, seed_vectors, w_q, w_k, w_v, w1, b1, w2, b2):
    edge_features = np.asarray(edge_features, dtype=np.float32)
    batch = np.asarray(batch)
    assign, slot_tiles, sizes = plan_shards(batch)
    TT = sum(slot_tiles)
    EC = TT * TILE

    starts = np.searchsorted(batch, np.arange(B))
    xb = edge_features.astype(BF16)

    # Ws[hin, h*S+s] = sum_d w_k[hin, h*HD+d] * q[s, h, d] / sqrt(HD)
    q = (np.asarray(seed_vectors, np.float32) @ np.asarray(w_q, np.float32)).reshape(
        S, NH, HD
    )
    wk3 = np.asarray(w_k, np.float32).reshape(H, NH, HD)
    Ws = (np.einsum("ihd,shd->ihs", wk3, q) * SCALE).reshape(H, NH * S)
    wvs = np.concatenate([np.asarray(w_v, np.float32), Ws], axis=1)

    shared = {
        "wvs": np.ascontiguousarray(wvs.astype(BF16)),
        "w1": np.ascontiguousarray(np.asarray(w1).astype(BF16)),
        "w2": np.ascontiguousarray(np.asarray(w2).astype(BF16)),
        "b1": np.ascontiguousarray(
            np.broadcast_to(np.asarray(b1, dtype=np.float32), (NG, H))
        ),
        "b2": np.ascontiguousarray(np.asarray(b2, dtype=np.float32).reshape(H, 1)),
        "ident": np.eye(128, dtype=BF16),
        "qsel": np.ascontiguousarray(
            (np.arange(128)[:, None] % 32 == np.arange(NG)[None, :]).astype(BF16)
        ),
    }

    in_maps = []
    for c in range(NCORES):
        xt = np.zeros((H, EC), dtype=BF16)
        npad = np.zeros(NG, dtype=np.float32)
        off = 0
        for j, g in enumerate(assign[c]):
            n = int(sizes[g])
            xt[:, off : off + n] = xb[starts[g] : starts[g] + n].T
            npad[j] = slot_tiles[j] * TILE - n
            off += slot_tiles[j] * TILE
        m = dict(shared)
        m["xt"] = xt
        m["npad"] = np.ascontiguousarray(np.broadcast_to(npad, (128, NG)))
        in_maps.append(m)
    return in_maps, assign, slot_tiles


def kernel(
    edge_features,
    edge_coords,
    batch,
    seed_vectors,
    w_q,
    w_k,
    w_v,
    w1,
    b1,
    w2,
    b2,
):
    in_maps, assign, slot_tiles = make_in_maps(
        edge_features, batch, seed_vectors, w_q, w_k, w_v, w1, b1, w2, b2
    )
    nc = get_program(slot_tiles)

    res = run_bass_kernel_spmd(nc, in_maps, core_ids=list(range(NCORES)))
    global LAST_RESULTS
    LAST_RESULTS = res

    out = np.zeros((B, H), dtype=np.float32)
    for c in range(NCORES):
        outT = res.results[c]["outT"]  # [H, NG]
        for j, g in enumerate(assign[c]):
            out[g, :] = outT[:, j]
    return out



# revision 12
# speedup vs baseline: 1.2985x; 1.2985x over previous
"""AttentionPooling (ragged graph cross-attention pooling) on 8 TRN2 NeuronCores.

Strategy (SPMD, no collectives):
  * Host assigns 8 whole graphs to each of the 8 cores (serpentine by size),
    sorts each core's graphs by size into 8 "slots".  Slot j has a fixed tile
    count T[j] (shared by all cores, since the instruction stream is shared);
    each graph's edges are placed at its slot offset and zero-padded.
  * Host ships x^T (transposed edge features, bf16) per core + replicated
    weights.  Padding edges give exp(0)=1 in the softmax denominator, which is
    corrected with a host-computed per-slot pad count.
  * Softmax is computed without max-subtraction (scores ~ N(0,1); exp cannot
    overflow fp32) — mathematically identical to the reference's stable form.
  * Scores are linear in x: scores = (x @ w_k) . q  =  x @ Ws where
    Ws[:, (h,s)] = sum_d w_k[:, (h,d)] q[s,h,d] / sqrt(hd).  Ws ([256, 256])
    is host-precomputed from the weights and shipped fused with w_v as one
    [256, 512] operand, so the per-tile device work is:
      [v | sc][e, :] = x @ [w_v | Ws]    (PE, 2 matmuls/tile, N=512)
      ex             = exp(sc)           (ACT, psum->sbuf bf16)
      pooled[(h,s),(h,d)|denom] += ex.T @ [v | 1]  (PE, psum-accum per graph)
  * Per graph: denom -= npad; normalize by 1/denom (DVE); 32x32 block
    transpose (DVE StreamTranspose) to build the [128, (s,half)*8graphs]
    operand P2 for the MLP (w1 needs no permutation in this layout).
  * MLP: h1 = silu(pooled @ w1 + b1) (PE, 4-way tile_position-packed, +ACT),
    out = h1 @ w2 + b2 (PE), emitted as out^T [256, 8] per core; the host
    scatters core outputs into the final [64, 256].
"""

import os
import sys
from contextlib import ExitStack

import numpy as np

for _p in ("/opt/trn_rl_repo",):
    if _p not in sys.path:
        sys.path.append(_p)

import ml_dtypes  # noqa: E402

import concourse.bass as bass  # noqa: E402
import concourse.tile as tile  # noqa: E402
from concourse import mybir  # noqa: E402
from concourse.bass_utils import run_bass_kernel_spmd  # noqa: E402
from concourse.vector_clock import ScopedClock  # noqa: E402

BF16 = ml_dtypes.bfloat16

E, B, H, S, NH, HD = 131072, 64, 256, 32, 8, 32
NCORES = 8
NG = B // NCORES        # graphs (slots) per core
TILE = 128              # edge tile
GROUP = 512             # x^T DMA chunk (4 tiles)
SCALE = 1.0 / float(np.sqrt(HD))

AF = mybir.ActivationFunctionType

# ---------------------------------------------------------------------------
# Walrus workaround: this toolchain's InstDrain accepts only ONE sync wait;
# Tile's kernel-tail drain carries one wait per outstanding semaphore.
# Split it into a chain of single-wait drains.
_MAXW = 1


def _split_drain_and_barrier(self, tick_clock, wait_clock):
    nc = self.nc
    drain_inst = nc.sync.drain()
    wait_clock.add_sem_waits(
        drain_inst.ins, ScopedClock({None: tick_clock.global_clock})
    )
    waits = list(drain_inst.ins.sync_info.on_wait)
    if len(waits) > _MAXW:
        drain_inst.ins.sync_info = mybir.SyncInfo(on_wait=waits[:_MAXW], on_update=[])
        for i in range(_MAXW, len(waits), _MAXW):
            d2 = nc.sync.drain()
            d2.ins.sync_info = mybir.SyncInfo(
                on_wait=waits[i : i + _MAXW], on_update=[]
            )
    popped = nc._tile_sem_poison_stack.pop()
    assert popped is self._sem_poison
    # Program runs once per NEFF execution; skip the per-semaphore clear
    # chain (several us of EVENT_SEMAPHORE resets) and settle for one
    # barrier.  Free the handles bookkeeping-only so later allocations
    # (none today) stay consistent.
    nc._state.prepend_free_semaphores(
        [getattr(s, "num", s) for s in self.sems.allocated().values()]
    )
    nc.all_engine_barrier()


tile.TileContext._drain_and_barrier = _split_drain_and_barrier

# Engine instructions are capped at 2 sync waits by this walrus (Drain/NoOp
# at 1).  Tile's sem-assignment occasionally emits more.  Hoist the excess
# onto single-wait NoOps inserted just before, on the same engine — the
# engine stalls at the NoOp instead, which is semantically identical.
_WAIT_CAP = {"InstDrain": 1}
_WAIT_CAP_DEFAULT = 1


def _fix_excess_waits(nc):
    n_fixed = 0
    for fn in nc.m.functions:
        for bb in fn.blocks:
            insts = bb.instructions
            out = []
            changed = False
            for inst in insts:
                si = inst.sync_info
                waits = list(si.on_wait) if si is not None else []
                cap = _WAIT_CAP.get(type(inst).__name__, _WAIT_CAP_DEFAULT)
                if len(waits) > cap:
                    changed = True
                    n_fixed += 1
                    excess = waits[: len(waits) - cap]
                    for i, w in enumerate(excess):
                        nop = mybir.InstNoOp(
                            name=f"{inst.name}-hw{i}", ins=[], outs=[]
                        )
                        nop.engine = inst.engine
                        nop.sync_info = mybir.SyncInfo(on_wait=[w], on_update=[])
                        out.append(nop)
                    inst.sync_info = mybir.SyncInfo(
                        on_wait=waits[len(excess) :], on_update=list(si.on_update)
                    )
                out.append(inst)
            if changed:
                bb.instructions = out
    return n_fixed

# ---------------------------------------------------------------------------

_PROGRAM_CACHE: dict[tuple, "bass.Bass"] = {}
LAST_RESULTS = None  # BassKernelResults of the most recent run (for testing)


def _install_ntff_hook_shim():
    """The image's antenv lacks axon_hooks; recreate it so trace=True works."""
    try:
        import types

        import antenv

        if "antenv.axon_hooks" not in sys.modules:
            mod = types.ModuleType("antenv.axon_hooks")
            mod._hook = None

            def set_axon_ntff_profile_hook(h):
                mod._hook = h

            def get_axon_ntff_profile_hook():
                return mod._hook

            mod.set_axon_ntff_profile_hook = set_axon_ntff_profile_hook
            mod.get_axon_ntff_profile_hook = get_axon_ntff_profile_hook
            sys.modules["antenv.axon_hooks"] = mod
            antenv.axon_hooks = mod
        import antenv.axon_hooks as ah

        if ah.get_axon_ntff_profile_hook() is None:
            from trn_agent_boot.trn_boot import _ntff_profile_via_ctypes

            ah.set_axon_ntff_profile_hook(
                _ntff_profile_via_ctypes("/opt/axon/libaxon_pjrt.so")
            )
    except Exception:
        pass


_install_ntff_hook_shim()

# Optional experiment: let walrus double-buffer LDWEIGHTS (default off here).
import concourse.bass_utils as _bass_utils  # noqa: E402

_orig_run_command = _bass_utils.run_command


def _run_command_ldwopt(cmd, **kw):
    if isinstance(cmd, list):
        cmd = [
            "--enable-ldw-opt=true" if c == "--enable-ldw-opt=false" else c
            for c in cmd
        ]
    return _orig_run_command(cmd, **kw)


if os.environ.get("KERNEL_LDW_OPT") == "1":
    _bass_utils.run_command = _run_command_ldwopt


def build_program(slot_tiles: tuple[int, ...]) -> "bass.Bass":
    """Build the SPMD Bass program for per-core slot tile counts."""
    TT = sum(slot_tiles)
    EC = TT * TILE
    assert TT % (GROUP // TILE) == 0
    NGRP = TT // (GROUP // TILE)

    # per-tile slot id / first / last flags
    slot_of, first_of, last_of = [], [], []
    for j, tj in enumerate(slot_tiles):
        for t in range(tj):
            slot_of.append(j)
            first_of.append(t == 0)
            last_of.append(t == tj - 1)

    f32, bf16 = mybir.dt.float32, mybir.dt.bfloat16
    nc = bass.Bass("TRN2", target_bir_lowering=False, debug=False, num_devices=NCORES)

    xt_d = nc.dram_tensor("xt", [H, EC], bf16, kind="ExternalInput").ap()
    wvs_d = nc.dram_tensor("wvs", [H, 2 * H], bf16, kind="ExternalInput").ap()
    w1_d = nc.dram_tensor("w1", [S * H, H], bf16, kind="ExternalInput").ap()
    w2_d = nc.dram_tensor("w2", [H, H], bf16, kind="ExternalInput").ap()
    b1_d = nc.dram_tensor("b1", [NG, H], f32, kind="ExternalInput").ap()
    b2_d = nc.dram_tensor("b2", [H, 1], f32, kind="ExternalInput").ap()
    npad_d = nc.dram_tensor("npad", [128, NG], f32, kind="ExternalInput").ap()
    ident_d = nc.dram_tensor("ident", [128, 128], bf16, kind="ExternalInput").ap()
    qsel_d = nc.dram_tensor("qsel", [128, NG], bf16, kind="ExternalInput").ap()
    outT_d = nc.dram_tensor("outT", [H, NG], f32, kind="ExternalOutput").ap()

    with tile.TileContext(nc) as tc, ExitStack() as ctx:
        const = ctx.enter_context(tc.tile_pool(name="const", bufs=1))
        w2_sb = const.tile([128, 2 * H], bf16)
        wvs_sb = const.tile([128, 2 * 2 * H], bf16)  # k-tile k: [wv_k | ws_k]
        w1_sb = const.tile([128, 64 * H], bf16)
        ident_sb = const.tile([128, 128], bf16)
        qsel_sb = const.tile([128, NG], bf16)
        b1_sb = const.tile([NG, H], f32)
        b2_sb = const.tile([128, 2], f32)
        npad_sb = const.tile([128, NG], f32)
        P2 = const.tile([128, 64 * NG], bf16)
        h1ps = const.tile([128, H], bf16, name="h1ps")
        nc.gpsimd.memset(h1ps[:], 0.0)

        # Warm the exp ACT table FIRST: the scalar engine must not issue
        # anything before this, so the (1.5us) table load overlaps the
        # startup DMAs instead of trailing them.  The whole kernel uses
        # only table-0 functions (Exp/Copy/Identity) — no reloads ever.
        warm = const.tile([1, 2], f32)
        nc.vector.memset(warm[:, 0:1], 0.0)
        nc.scalar.activation(warm[:, 1:2], warm[:, 0:1], AF.Exp)

        # Const DMAs ride the gpsimd queue: scalar stays free for exp,
        # sync stays free for the xt stream.  wvs goes first — the first
        # vsc matmul blocks on it.
        for k in range(2):
            r = slice(k * 128, (k + 1) * 128)
            nc.gpsimd.dma_start(wvs_sb[:, k * 2 * H : (k + 1) * 2 * H], wvs_d[r, :])
        nc.gpsimd.dma_start(npad_sb[:], npad_d[:])
        for k in range(2):
            r = slice(k * 128, (k + 1) * 128)
            nc.gpsimd.dma_start(w2_sb[:, k * H : (k + 1) * H], w2_d[r, :])
            nc.gpsimd.dma_start(b2_sb[:, k : k + 1], b2_d[r, :])
        nc.gpsimd.dma_start(ident_sb[:], ident_d[:])
        nc.gpsimd.dma_start(qsel_sb[:], qsel_d[:])
        nc.gpsimd.dma_start(b1_sb[:], b1_d[:])
        # w1 (4MB) early on the gpsimd ring: arrives mid-loop, long before
        # the MLP tail needs it, without touching the sync/scalar rings.
        nc.gpsimd.dma_start(
            w1_sb[:].rearrange("p (k c) -> p k c", c=H),
            w1_d[:].rearrange("(k p) c -> p k c", p=128),
        )

        # ---- main edge loop ---------------------------------------------
        xt_pool = ctx.enter_context(tc.tile_pool(name="xtp", bufs=5))
        ex_pool = ctx.enter_context(tc.tile_pool(name="exp", bufs=8))
        ext_pool = ctx.enter_context(tc.tile_pool(name="ext", bufs=3))

        NRING = 8
        vs_ring = [const.tile([128, 258], bf16, name=f"vsring{i}") for i in range(NRING)]
        for t in vs_ring:
            nc.vector.memset(t[:, 128:129], 1.0)
            nc.vector.memset(t[:, 257:258], 1.0)

        pooled_tiles: list = [None, None]

        def emit_pooled(sl, fi, la, ex, vs):
            if fi:
                pooled_tiles[0] = pl_pool.tile([128, 129], f32, tag="pl0", name=f"pl0_s{sl}")
                pooled_tiles[1] = pl_pool.tile([128, 129], f32, tag="pl1", name=f"pl1_s{sl}")
            for m in range(2):
                nc.tensor.matmul(
                    pooled_tiles[m][:],
                    ex[:, m * 128 : (m + 1) * 128],
                    vs[:, m * 129 : m * 129 + 129],
                    start=fi,
                    stop=la,
                )
            if la:
                extract_graph(sl, pooled_tiles)

        # P2 layout: column g*64 + m*32 + s  (graph-major, then (half, seed)).
        # Extract writes CONTIGUOUS 32-col runs (fast DVE/Pool path); the MLP
        # reads graph-strided [128, NG] blocks via the LDWEIGHTS pattern.
        P2v = P2[:].rearrange("p (g j) -> p g j", j=64)

        def extract_graph(g, ptiles):
            copy_eng = nc.vector if g == NG - 1 else nc.gpsimd
            for m in range(2):
                den = ext_pool.tile([128, 1], f32, tag="den", name=f"den{g}_{m}")
                nc.vector.tensor_scalar_sub(
                    den[:], ptiles[m][:, 128:129], npad_sb[:, g : g + 1]
                )
                rec = ext_pool.tile([128, 1], f32, tag="rec", name=f"rec{g}_{m}")
                nc.vector.reciprocal(rec[:], den[:])
                # normalize on the ACT engine (Copy with per-partition scale)
                # straight out of PSUM into bf16 — keeps the DVE free for the
                # per-tile v copies; bf16 halves the StreamTranspose cost.
                pn = ext_pool.tile([128, 128], bf16, tag="pn", name=f"pn{g}_{m}")
                nc.scalar.activation(
                    pn[:], ptiles[m][:, 0:128], AF.Copy, scale=rec[:]
                )
                pt = ext_pool.tile([128, 128], bf16, tag="pt", name=f"pt{g}_{m}")
                nc.vector.transpose(pt[:], pn[:])
                for hh in range(4):
                    rr = slice(hh * 32, (hh + 1) * 32)
                    src = pt[rr, hh * 32 : (hh + 1) * 32].rearrange(
                        "p (a o) -> p a o", o=1
                    )
                    copy_eng.tensor_copy(P2v[rr, :, m * NG + g : m * NG + g + 1], src)

        with (
            tc.tile_pool(name="vscp", bufs=4, space="PSUM") as vsc_pool,
            tc.tile_pool(name="plp", bufs=2, space="PSUM") as pl_pool,
        ):
            from collections import deque

            pending = deque()
            tidx = 0
            for grp in range(NGRP):
                xt = [
                    xt_pool.tile([128, GROUP], bf16, tag="xt", name=f"xt_{grp}_{i}")
                    for i in range(2)
                ]
                for k in range(2):
                    nc.sync.dma_start(
                        xt[k][:],
                        xt_d[k * 128 : (k + 1) * 128, grp * GROUP : (grp + 1) * GROUP],
                    )
                for sub in range(4):
                    sl, fi, la = slot_of[tidx], first_of[tidx], last_of[tidx]
                    e0 = sub * TILE
                    vsc = vsc_pool.tile([128, 512], f32, tag="vsc", name=f"vsc{tidx}")
                    for k in range(2):
                        nc.tensor.matmul(
                            vsc[:],
                            xt[k][:, e0 : e0 + TILE],
                            wvs_sb[:, k * 2 * H : (k + 1) * 2 * H],
                            start=(k == 0),
                            stop=(k == 1),
                        )
                    ex = ex_pool.tile([128, 256], bf16, tag="ex", name=f"ex{tidx}")
                    nc.scalar.activation(ex[:], vsc[:, H : 2 * H], AF.Exp)
                    vs = vs_ring[tidx % NRING]
                    nc.vector.tensor_copy(
                        vs[:].rearrange("p (b c) -> p b c", c=129)[:, :, 0:128],
                        vsc[:, 0:H].rearrange("p (b c) -> p b c", c=128),
                    )
                    pending.append((sl, fi, la, ex, vs))
                    while len(pending) > 3:
                        emit_pooled(*pending.popleft())
                    tidx += 1
            while pending:
                emit_pooled(*pending.popleft())

        # ---- MLP tail ----------------------------------------------------
        with (
            tc.tile_pool(name="mlpp", bufs=2, space="PSUM") as mp,
            tc.tile_pool(name="mlps", bufs=2) as ms,
        ):
            h1pp = mp.tile([128, H], f32, tag="h1pp")
            for j in range(64):
                q = j % 4
                nc.tensor.matmul(
                    h1pp[q * 32 : q * 32 + NG, :],
                    P2[:, j * NG : (j + 1) * NG],
                    w1_sb[:, j * H : (j + 1) * H],
                    start=(j < 4),
                    stop=(j >= 60),
                    tile_position=(0, q * 32),
                    skip_group_check=True,
                )
            for q in range(4):
                eng = nc.vector if q % 2 == 0 else nc.scalar
                if eng is nc.vector:
                    nc.vector.tensor_copy(
                        h1ps[q * 32 : q * 32 + NG, :], h1pp[q * 32 : q * 32 + NG, :]
                    )
                else:
                    nc.scalar.activation(
                        h1ps[q * 32 : q * 32 + NG, :],
                        h1pp[q * 32 : q * 32 + NG, :],
                        AF.Copy,
                    )
            h1p = mp.tile([NG, H], f32, tag="h1p")
            nc.tensor.matmul(h1p[:], qsel_sb[:], h1ps[:], start=True, stop=True)
            h1s = ms.tile([NG, H], f32, tag="h1s")
            nc.vector.tensor_add(h1s[:], h1p[:], b1_sb[:])
            # silu(x) = x / (1 + exp(-x)) — stays on the exp ACT table
            # (Sigmoid would trigger a 1.5us ACT_TABLE_LOAD on the tail).
            h1e = ms.tile([NG, H], f32, tag="h1e")
            nc.scalar.activation(h1e[:], h1s[:], AF.Exp, scale=-1.0)
            h1d = ms.tile([NG, H], f32, tag="h1d")
            nc.vector.tensor_scalar_add(h1d[:], h1e[:], 1.0)
            h1r = ms.tile([NG, H], f32, tag="h1r")
            nc.vector.reciprocal(h1r[:], h1d[:])
            h1b = ms.tile([NG, H], bf16, tag="h1b")
            nc.vector.tensor_mul(h1b[:], h1s[:], h1r[:])
            h1t = []
            for m in range(2):
                h1tp = mp.tile([128, NG], bf16, tag="h1tp", name=f"h1tp{m}")
                nc.tensor.transpose(
                    h1tp[:], h1b[:, m * 128 : (m + 1) * 128], ident_sb[0:NG, 0:NG]
                )
                ht = ms.tile([128, NG], bf16, tag=f"h1t{m}")
                nc.vector.tensor_copy(ht[:], h1tp[:])
                h1t.append(ht)
            for m in range(2):
                otp = mp.tile([128, NG], f32, tag="otp", name=f"otp{m}")
                for k in range(2):
                    nc.tensor.matmul(
                        otp[:],
                        w2_sb[:, k * H + m * 128 : k * H + m * 128 + 128],
                        h1t[k][:],
                        start=(k == 0),
                        stop=(k == 1),
                    )
                osb = ms.tile([128, NG], f32, tag="osb", name=f"osb{m}")
                nc.vector.tensor_scalar_add(osb[:], otp[:], b2_sb[:, m : m + 1])
                nc.sync.dma_start(outT_d[m * 128 : (m + 1) * 128, :], osb[:])

    return nc


def get_program(slot_tiles: tuple[int, ...]) -> "bass.Bass":
    if slot_tiles not in _PROGRAM_CACHE:
        nc = build_program(slot_tiles)
        # HW-path only (CoreSim snapshots the program before this pass)
        _fix_excess_waits(nc)
        _PROGRAM_CACHE[slot_tiles] = nc
    return _PROGRAM_CACHE[slot_tiles]


# ---------------------------------------------------------------------------
# Host-side sharding / padding


def plan_shards(batch: np.ndarray):
    """Returns (assign [NCORES][NG] graph ids, slot_tiles tuple, sizes)."""
    sizes = np.bincount(batch, minlength=B).astype(np.int64)
    order = np.argsort(-sizes, kind="stable")
    assign = [[] for _ in range(NCORES)]
    for r in range(NG):
        row = order[r * NCORES : (r + 1) * NCORES]
        if r % 2 == 1:
            row = row[::-1]
        for c in range(NCORES):
            assign[c].append(int(row[c]))
    for c in range(NCORES):
        assign[c].sort(key=lambda g: -sizes[g])
    slot_tiles = []
    for j in range(NG):
        mx = max(sizes[assign[c][j]] for c in range(NCORES))
        slot_tiles.append(int(max(1, -(-mx // TILE))))
    # round total tiles up to a GROUP multiple (pad goes to the last slot)
    rem = (-sum(slot_tiles)) % (GROUP // TILE)
    slot_tiles[-1] += rem
    return assign, tuple(slot_tiles), sizes


def make_in_maps(edge_features, batch, seed_vectors, w_q, w_k, w_v, w1, b1, w2, b2):
    edge_features = np.asarray(edge_features, dtype=np.float32)
    batch = np.asarray(batch)
    assign, slot_tiles, sizes = plan_shards(batch)
    TT = sum(slot_tiles)
    EC = TT * TILE

    starts = np.searchsorted(batch, np.arange(B))
    xb = edge_features.astype(BF16)

    # Ws[hin, h*S+s] = sum_d w_k[hin, h*HD+d] * q[s, h, d] / sqrt(HD)
    q = (np.asarray(seed_vectors, np.float32) @ np.asarray(w_q, np.float32)).reshape(
        S, NH, HD
    )
    wk3 = np.asarray(w_k, np.float32).reshape(H, NH, HD)
    Ws = (np.einsum("ihd,shd->ihs", wk3, q) * SCALE).reshape(H, NH * S)
    wvs = np.concatenate([np.asarray(w_v, np.float32), Ws], axis=1)

    shared = {
        "wvs": np.ascontiguousarray(wvs.astype(BF16)),
        "w1": np.ascontiguousarray(np.asarray(w1).astype(BF16)),
        "w2": np.ascontiguousarray(np.asarray(w2).astype(BF16)),
        "b1": np.ascontiguousarray(
            np.broadcast_to(np.asarray(b1, dtype=np.float32), (NG, H))
        ),
        "b2": np.ascontiguousarray(np.asarray(b2, dtype=np.float32).reshape(H, 1)),
        "ident": np.eye(128, dtype=BF16),
        "qsel": np.ascontiguousarray(
            (np.arange(128)[:, None] % 32 == np.arange(NG)[None, :]).astype(BF16)
        ),
    }

    in_maps = []
    for c in range(NCORES):
        xt = np.zeros((H, EC), dtype=BF16)
        npad = np.zeros(NG, dtype=np.float32)
        off = 0
        for j, g in enumerate(assign[c]):
            n = int(sizes[g])
            xt[:, off : off + n] = xb[starts[g] : starts[g] + n].T
            npad[j] = slot_tiles[j] * TILE - n
            off += slot_tiles[j] * TILE
        m = dict(shared)
        m["xt"] = xt
        m["npad"] = np.ascontiguousarray(np.broadcast_to(npad, (128, NG)))
        in_maps.append(m)
    return in_maps, assign, slot_tiles


def kernel(
    edge_features,
    edge_coords,
    batch,
    seed_vectors,
    w_q,
    w_k,
    w_v,
    w1,
    b1,
    w2,
    b2,
):
    in_maps, assign, slot_tiles = make_in_maps(
        edge_features, batch, seed_vectors, w_q, w_k, w_v, w1, b1, w2, b2
    )
    nc = get_program(slot_tiles)

    res = run_bass_kernel_spmd(nc, in_maps, core_ids=list(range(NCORES)))
    global LAST_RESULTS
    LAST_RESULTS = res

    out = np.zeros((B, H), dtype=np.float32)
    for c in range(NCORES):
        outT = res.results[c]["outT"]  # [H, NG]
        for j, g in enumerate(assign[c]):
            out[g, :] = outT[:, j]
    return out



# revision 38
# speedup vs baseline: 1.6059x; 1.2368x over previous
"""AttentionPooling (ragged graph cross-attention pooling) on 8 TRN2 NeuronCores.

Strategy (SPMD, no collectives):
  * Host assigns 8 whole graphs to each of the 8 cores (serpentine by size),
    sorts each core's graphs by size into 8 "slots".  Slot j has a fixed tile
    count T[j] (shared by all cores, since the instruction stream is shared);
    each graph's edges are placed at its slot offset and zero-padded.
  * Host ships x^T (transposed edge features, bf16) per core + replicated
    weights.  Padding edges give exp(0)=1 in the softmax denominator, which is
    corrected with a host-computed per-slot pad count.
  * Softmax is computed without max-subtraction (scores ~ N(0,1); exp cannot
    overflow fp32) — mathematically identical to the reference's stable form.
  * Scores are linear in x: scores = (x @ w_k) . q  =  x @ Ws where
    Ws[:, (h,s)] = sum_d w_k[:, (h,d)] q[s,h,d] / sqrt(hd).  Ws ([256, 256])
    is host-precomputed from the weights and shipped fused with w_v as one
    [256, 512] operand, so the per-tile device work is:
      [v | sc][e, :] = x @ [w_v | Ws]    (PE, 2 matmuls/tile, N=512)
      ex             = exp(sc)           (ACT, psum->sbuf bf16)
      pooled[(h,s),(h,d)|denom] += ex.T @ [v | 1]  (PE, psum-accum per graph)
  * Per graph: denom -= npad; recip (DVE); normalize out of PSUM into bf16
    (ACT Copy w/ scale for m=0, DVE for m=1); 32x32 block transpose (DVE
    StreamTranspose); packed 32-col copies into P2, laid out graph-major
    [128, g*64 + (half*32+seed)] so extract writes are contiguous and the
    MLP reads graph-strided [128, 8] blocks via the LDWEIGHTS pattern (w1
    is host-permuted to matching (half, seed)-major k-blocks).
  * MLP: h1pp = P2^T @ w1 (PE, 4-way tile_position col-group packing, the
    4 quadrant strips summed on DVE with b1), silu via x*0.5*(1+tanh(x/2))
    (tanh shares the exp ACT table - no mid-kernel table reload; the 0.5 is
    folded into w2 host-side), out = h1 @ w2' + b2 (PE) emitted as out^T
    [256, 8] per core; the host scatters core outputs into [64, 256].
  * Schedule notes: exp table warms before anything else on ACT; const DMAs
    ride gpsimd; tail-only consts + w1 (in 8 thin slices) are pinned behind
    mid-loop ex tiles via 1-elem dummy copies so the Tile scheduler cannot
    hoist their transfers into the xt ramp-up; xt_pool bufs=10 covers the
    DMA bandwidth-delay product; dependency-free filler matmuls bridge the
    loop-drain and extract gaps so the PE p-state clock stays at full rate
    into the MLP; the Tile teardown skips the per-semaphore clear chain
    (one barrier only) since the NEFF runs once per launch.
"""

import os
import sys
from contextlib import ExitStack

import numpy as np

for _p in ("/opt/trn_rl_repo",):
    if _p not in sys.path:
        sys.path.append(_p)

import ml_dtypes  # noqa: E402

import concourse.bass as bass  # noqa: E402
import concourse.tile as tile  # noqa: E402
from concourse import mybir  # noqa: E402
from concourse.bass_utils import run_bass_kernel_spmd  # noqa: E402
from concourse.vector_clock import ScopedClock  # noqa: E402

BF16 = ml_dtypes.bfloat16

E, B, H, S, NH, HD = 131072, 64, 256, 32, 8, 32
NCORES = 8
NG = B // NCORES        # graphs (slots) per core
TILE = 128              # edge tile
GROUP = 512             # x^T DMA chunk (4 tiles)
SCALE = 1.0 / float(np.sqrt(HD))

AF = mybir.ActivationFunctionType

# ---------------------------------------------------------------------------
# Walrus workaround: this toolchain's InstDrain accepts only ONE sync wait;
# Tile's kernel-tail drain carries one wait per outstanding semaphore.
# Split it into a chain of single-wait drains.
_MAXW = 1


def _split_drain_and_barrier(self, tick_clock, wait_clock):
    nc = self.nc
    drain_inst = nc.sync.drain()
    wait_clock.add_sem_waits(
        drain_inst.ins, ScopedClock({None: tick_clock.global_clock})
    )
    waits = list(drain_inst.ins.sync_info.on_wait)
    if len(waits) > _MAXW:
        drain_inst.ins.sync_info = mybir.SyncInfo(on_wait=waits[:_MAXW], on_update=[])
        for i in range(_MAXW, len(waits), _MAXW):
            d2 = nc.sync.drain()
            d2.ins.sync_info = mybir.SyncInfo(
                on_wait=waits[i : i + _MAXW], on_update=[]
            )
    popped = nc._tile_sem_poison_stack.pop()
    assert popped is self._sem_poison
    # Program runs once per NEFF execution; skip the per-semaphore clear
    # chain (several us of EVENT_SEMAPHORE resets) and settle for one
    # barrier.  Free the handles bookkeeping-only so later allocations
    # (none today) stay consistent.
    nc._state.prepend_free_semaphores(
        [getattr(s, "num", s) for s in self.sems.allocated().values()]
    )
    nc.all_engine_barrier()


tile.TileContext._drain_and_barrier = _split_drain_and_barrier

# Engine instructions are capped at 2 sync waits by this walrus (Drain/NoOp
# at 1).  Tile's sem-assignment occasionally emits more.  Hoist the excess
# onto single-wait NoOps inserted just before, on the same engine — the
# engine stalls at the NoOp instead, which is semantically identical.
_WAIT_CAP = {"InstDrain": 1}
_WAIT_CAP_DEFAULT = 1


def _fix_excess_waits(nc):
    n_fixed = 0
    for fn in nc.m.functions:
        for bb in fn.blocks:
            insts = bb.instructions
            out = []
            changed = False
            for inst in insts:
                si = inst.sync_info
                waits = list(si.on_wait) if si is not None else []
                cap = _WAIT_CAP.get(type(inst).__name__, _WAIT_CAP_DEFAULT)
                if len(waits) > cap:
                    changed = True
                    n_fixed += 1
                    excess = waits[: len(waits) - cap]
                    for i, w in enumerate(excess):
                        nop = mybir.InstNoOp(
                            name=f"{inst.name}-hw{i}", ins=[], outs=[]
                        )
                        nop.engine = inst.engine
                        nop.sync_info = mybir.SyncInfo(on_wait=[w], on_update=[])
                        out.append(nop)
                    inst.sync_info = mybir.SyncInfo(
                        on_wait=waits[len(excess) :], on_update=list(si.on_update)
                    )
                out.append(inst)
            if changed:
                bb.instructions = out
    return n_fixed

# ---------------------------------------------------------------------------

_PROGRAM_CACHE: dict[tuple, "bass.Bass"] = {}
LAST_RESULTS = None  # BassKernelResults of the most recent run (for testing)


def _install_ntff_hook_shim():
    """The image's antenv lacks axon_hooks; recreate it so trace=True works."""
    try:
        import types

        import antenv

        if "antenv.axon_hooks" not in sys.modules:
            mod = types.ModuleType("antenv.axon_hooks")
            mod._hook = None

            def set_axon_ntff_profile_hook(h):
                mod._hook = h

            def get_axon_ntff_profile_hook():
                return mod._hook

            mod.set_axon_ntff_profile_hook = set_axon_ntff_profile_hook
            mod.get_axon_ntff_profile_hook = get_axon_ntff_profile_hook
            sys.modules["antenv.axon_hooks"] = mod
            antenv.axon_hooks = mod
        import antenv.axon_hooks as ah

        if ah.get_axon_ntff_profile_hook() is None:
            from trn_agent_boot.trn_boot import _ntff_profile_via_ctypes

            ah.set_axon_ntff_profile_hook(
                _ntff_profile_via_ctypes("/opt/axon/libaxon_pjrt.so")
            )
    except Exception:
        pass


_install_ntff_hook_shim()

# Optional experiment: let walrus double-buffer LDWEIGHTS (default off here).
import concourse.bass_utils as _bass_utils  # noqa: E402

_orig_run_command = _bass_utils.run_command


def _run_command_ldwopt(cmd, **kw):
    if isinstance(cmd, list):
        cmd = [
            "--enable-ldw-opt=true" if c == "--enable-ldw-opt=false" else c
            for c in cmd
        ]
    return _orig_run_command(cmd, **kw)


if os.environ.get("KERNEL_LDW_OPT") == "1":
    _bass_utils.run_command = _run_command_ldwopt


def build_program(slot_tiles: tuple[int, ...]) -> "bass.Bass":
    """Build the SPMD Bass program for per-core slot tile counts."""
    TT = sum(slot_tiles)
    EC = TT * TILE
    assert TT % (GROUP // TILE) == 0
    NGRP = TT // (GROUP // TILE)

    # per-tile slot id / first / last flags
    slot_of, first_of, last_of = [], [], []
    for j, tj in enumerate(slot_tiles):
        for t in range(tj):
            slot_of.append(j)
            first_of.append(t == 0)
            last_of.append(t == tj - 1)

    f32, bf16 = mybir.dt.float32, mybir.dt.bfloat16
    nc = bass.Bass("TRN2", target_bir_lowering=False, debug=False, num_devices=NCORES)

    xt_d = nc.dram_tensor("xt", [H, EC], bf16, kind="ExternalInput").ap()
    wvs_d = nc.dram_tensor("wvs", [H, 2 * H], bf16, kind="ExternalInput").ap()
    w1_d = nc.dram_tensor("w1", [S * H, H], bf16, kind="ExternalInput").ap()
    w2_d = nc.dram_tensor("w2", [H, H], bf16, kind="ExternalInput").ap()
    b1_d = nc.dram_tensor("b1", [NG, H], f32, kind="ExternalInput").ap()
    b2_d = nc.dram_tensor("b2", [H, 1], f32, kind="ExternalInput").ap()
    npad_d = nc.dram_tensor("npad", [128, NG], f32, kind="ExternalInput").ap()
    ident_d = nc.dram_tensor("ident", [128, 128], bf16, kind="ExternalInput").ap()
    outT_d = nc.dram_tensor("outT", [H, NG], f32, kind="ExternalOutput").ap()

    with tile.TileContext(nc) as tc, ExitStack() as ctx:
        const = ctx.enter_context(tc.tile_pool(name="const", bufs=1))
        w2_sb = const.tile([128, 2 * H], bf16)
        wvs_sb = const.tile([128, 2 * 2 * H], bf16)  # k-tile k: [wv_k | ws_k]
        w1_sb = const.tile([128, 64 * H], bf16)
        ident_sb = const.tile([128, 128], bf16)
        b1_sb = const.tile([NG, H], f32)
        b2_sb = const.tile([128, 2], f32)
        npad_sb = const.tile([128, NG], f32)
        P2 = const.tile([128, 64 * NG], bf16)

        # Warm the exp ACT table FIRST: the scalar engine must not issue
        # anything before this, so the (1.5us) table load overlaps the
        # startup DMAs instead of trailing them.  The whole kernel uses
        # only table-0 functions (Exp/Copy/Identity) — no reloads ever.
        warm = const.tile([1, 2], f32)
        nc.vector.memset(warm[:, 0:1], 0.0)
        nc.scalar.activation(warm[:, 1:2], warm[:, 0:1], AF.Exp)

        # Const DMAs ride the gpsimd queue: scalar stays free for exp,
        # sync stays free for the xt stream.  wvs goes first — the first
        # vsc matmul blocks on it.
        for k in range(2):
            r = slice(k * 128, (k + 1) * 128)
            nc.gpsimd.dma_start(wvs_sb[:, k * 2 * H : (k + 1) * 2 * H], wvs_d[r, :])
        nc.gpsimd.dma_start(npad_sb[:], npad_d[:])

        # ---- main edge loop ---------------------------------------------
        xt_pool = ctx.enter_context(tc.tile_pool(name="xtp", bufs=10))
        ex_pool = ctx.enter_context(tc.tile_pool(name="exp", bufs=10))
        ext_pool = ctx.enter_context(tc.tile_pool(name="ext", bufs=3))

        NRING = 10
        vs_ring = [const.tile([128, 258], bf16, name=f"vsring{i}") for i in range(NRING)]
        for t in vs_ring:
            nc.vector.memset(t[:, 128:129], 1.0)
            nc.vector.memset(t[:, 257:258], 1.0)

        pooled_tiles: list = [None, None]

        def emit_pooled(sl, fi, la, ex, vs):
            if fi:
                pooled_tiles[0] = pl_pool.tile([128, 129], f32, tag="pl0", name=f"pl0_s{sl}")
                pooled_tiles[1] = pl_pool.tile([128, 129], f32, tag="pl1", name=f"pl1_s{sl}")
            for m in range(2):
                nc.tensor.matmul(
                    pooled_tiles[m][:],
                    ex[:, m * 128 : (m + 1) * 128],
                    vs[:, m * 129 : m * 129 + 129],
                    start=fi,
                    stop=la,
                )
            if la:
                extract_graph(sl, pooled_tiles)

        # P2 layout: column g*64 + m*32 + s  (graph-major, then (half, seed)).
        # Extract writes CONTIGUOUS 32-col runs (fast DVE/Pool path); the MLP
        # reads graph-strided [128, NG] blocks via the LDWEIGHTS pattern.
        P2v = P2[:].rearrange("p (g j) -> p g j", j=64)

        def extract_graph(g, ptiles):
            copy_eng = nc.vector if g == NG - 1 else nc.gpsimd
            for m in range(2):
                den = ext_pool.tile([128, 1], f32, tag="den", name=f"den{g}_{m}")
                nc.vector.tensor_scalar_sub(
                    den[:], ptiles[m][:, 128:129], npad_sb[:, g : g + 1]
                )
                rec = ext_pool.tile([128, 1], f32, tag="rec", name=f"rec{g}_{m}")
                nc.vector.reciprocal(rec[:], den[:])
                # normalize straight out of PSUM into bf16, split across the
                # ACT engine (m=0) and DVE (m=1) so neither engine eats the
                # whole slot-boundary burst.
                pn = ext_pool.tile([128, 128], bf16, tag="pn", name=f"pn{g}_{m}")
                if m == 0:
                    nc.scalar.activation(
                        pn[:], ptiles[m][:, 0:128], AF.Copy, scale=rec[:]
                    )
                else:
                    nc.vector.tensor_scalar_mul(pn[:], ptiles[m][:, 0:128], rec[:])
                pt = ext_pool.tile([128, 128], bf16, tag="pt", name=f"pt{g}_{m}")
                nc.vector.transpose(pt[:], pn[:])
                for hh in range(4):
                    rr = slice(hh * 32, (hh + 1) * 32)
                    copy_eng.tensor_copy(
                        P2v[rr, g, m * 32 : (m + 1) * 32],
                        pt[rr, hh * 32 : (hh + 1) * 32],
                    )

        with (
            tc.tile_pool(name="vscp", bufs=4, space="PSUM") as vsc_pool,
            tc.tile_pool(name="plp", bufs=2, space="PSUM") as pl_pool,
        ):
            from collections import deque

            pending = deque()
            tidx = 0
            for grp in range(NGRP):
                xt = [
                    xt_pool.tile([128, GROUP], bf16, tag="xt", name=f"xt_{grp}_{i}")
                    for i in range(2)
                ]
                for k in range(2):
                    nc.sync.dma_start(
                        xt[k][:],
                        xt_d[k * 128 : (k + 1) * 128, grp * GROUP : (grp + 1) * GROUP],
                    )
                for sub in range(4):
                    sl, fi, la = slot_of[tidx], first_of[tidx], last_of[tidx]
                    e0 = sub * TILE
                    vsc = vsc_pool.tile([128, 512], f32, tag="vsc", name=f"vsc{tidx}")
                    for k in range(2):
                        nc.tensor.matmul(
                            vsc[:],
                            xt[k][:, e0 : e0 + TILE],
                            wvs_sb[:, k * 2 * H : (k + 1) * 2 * H],
                            start=(k == 0),
                            stop=(k == 1),
                        )
                    ex = ex_pool.tile([128, 256], bf16, tag="ex", name=f"ex{tidx}")
                    nc.scalar.activation(ex[:], vsc[:, H : 2 * H], AF.Exp)
                    vs = vs_ring[tidx % NRING]
                    nc.vector.tensor_copy(
                        vs[:].rearrange("p (b c) -> p b c", c=129)[:, :, 0:128],
                        vsc[:, 0:H].rearrange("p (b c) -> p b c", c=128),
                    )
                    if tidx == 24:
                        # Tail-only consts must NOT transfer during the xt
                        # ramp-up.  The Tile scheduler hoists dependency-free
                        # DMAs to the stream front, so pin each behind this
                        # tile's ex via a 1-element dummy copy (the DMA then
                        # overwrites it with real data).
                        for tgt in (w2_sb, ident_sb):
                            nc.gpsimd.tensor_copy(tgt[0:1, 0:1], ex[0:1, 0:1])
                        for k in range(2):
                            r = slice(k * 128, (k + 1) * 128)
                            nc.gpsimd.dma_start(
                                w2_sb[:, k * H : (k + 1) * H], w2_d[r, :]
                            )
                            nc.gpsimd.dma_start(b2_sb[:, k : k + 1], b2_d[r, :])
                        nc.gpsimd.dma_start(ident_sb[:], ident_d[:])
                        nc.gpsimd.dma_start(b1_sb[:], b1_d[:])
                    if tidx >= 28 and tidx % 12 == 4 and (si := (tidx - 28) // 12) < 8:
                        # w1 (4MB) in 8 thin slices spread across the loop:
                        # ~512KB every ~12 tiles keeps its HBM draw far below
                        # the xt stream's needs.
                        nc.gpsimd.tensor_copy(
                            w1_sb[0:1, si * 8 * H : si * 8 * H + 1], ex[0:1, 0:1]
                        )
                        nc.gpsimd.dma_start(
                            w1_sb[:, si * 8 * H : (si + 1) * 8 * H].rearrange(
                                "p (k c) -> p k c", c=H
                            ),
                            w1_d[si * 1024 : (si + 1) * 1024, :].rearrange(
                                "(k p) c -> p k c", p=128
                            ),
                        )
                    pending.append((sl, fi, la, ex, vs))
                    while len(pending) > 5:
                        emit_pooled(*pending.popleft())
                    tidx += 1
            # Drain phase: pooled emissions now run at copy-latency rhythm
            # with 300-500ns PE gaps that would reset the PE p-state ramp
            # (halving the MLP's clock).  Interleave dependency-free filler
            # matmuls to keep the clock hot into the tail.
            scratch = vsc_pool.tile([128, 512], f32, tag="vsc", name="scratch")
            while pending:
                emit_pooled(*pending.popleft())
                for _ in range(2):
                    nc.tensor.matmul(
                        scratch[:, 0:256], ident_sb[:], wvs_sb[:, 0:256],
                        start=True, stop=True,
                    )

        # ---- MLP tail ----------------------------------------------------
        with (
            tc.tile_pool(name="mlpp", bufs=2, space="PSUM") as mp,
            tc.tile_pool(name="mlps", bufs=2) as ms,
        ):
            # Keep the PE p-state hot across the last-graph extract: a few
            # dependency-free filler matmuls bridge the idle gap so h1pp
            # streams at full clock instead of the half-rate ramp.
            warmmm = mp.tile([128, 256], f32, tag="warmmm")
            for wi in range(6):
                nc.tensor.matmul(
                    warmmm[:], ident_sb[:], wvs_sb[:, 0:256],
                    start=True, stop=True,
                )
            h1pp = mp.tile([128, H], f32, tag="h1pp")
            for j in range(64):
                q = j % 4
                nc.tensor.matmul(
                    h1pp[q * 32 : q * 32 + NG, :],
                    P2v[:, :, j],
                    w1_sb[:, j * H : (j + 1) * H],
                    start=(j < 4),
                    stop=(j >= 60),
                    tile_position=(0, q * 32),
                    skip_group_check=True,
                )
            # quadrant sum + b1 on DVE/Pool straight from PSUM (no qsel
            # matmul); only one PSUM operand per instruction is legal.
            # pure-vector accumulation chain: Tile's wait coalescing
            # serializes cross-engine PSUM readers anyway, so avoid the
            # extra engine hops entirely.
            t01 = ms.tile([NG, H], f32, tag="t01")
            nc.vector.tensor_add(t01[:], h1pp[0:NG, :], b1_sb[:])
            t02 = ms.tile([NG, H], f32, tag="t02")
            nc.vector.tensor_add(t02[:], h1pp[32 : 32 + NG, :], t01[:])
            t03 = ms.tile([NG, H], f32, tag="t03")
            nc.vector.tensor_add(t03[:], h1pp[64 : 64 + NG, :], t02[:])
            h1s = ms.tile([NG, H], f32, tag="h1s")
            nc.vector.tensor_add(h1s[:], h1pp[96 : 96 + NG, :], t03[:])
            # silu(x) = x * sigmoid(x) = 0.5*x*(1 + tanh(x/2)); tanh lives in
            # the exp ACT table (no reload) and the 0.5 is folded into w2
            # host-side.
            h1e = ms.tile([NG, H], f32, tag="h1e")
            nc.scalar.activation(h1e[:], h1s[:], AF.Tanh, scale=0.5)
            h1d = ms.tile([NG, H], f32, tag="h1d")
            nc.vector.tensor_scalar_add(h1d[:], h1e[:], 1.0)
            h1b = ms.tile([NG, H], bf16, tag="h1b")
            nc.vector.tensor_mul(h1b[:], h1s[:], h1d[:])
            h1t = []
            for m in range(2):
                h1tp = mp.tile([128, NG], bf16, tag="h1tp", name=f"h1tp{m}")
                nc.tensor.transpose(
                    h1tp[:], h1b[:, m * 128 : (m + 1) * 128], ident_sb[0:NG, 0:NG]
                )
                ht = ms.tile([128, NG], bf16, tag=f"h1t{m}")
                nc.vector.tensor_copy(ht[:], h1tp[:])
                h1t.append(ht)
            for m in range(2):
                otp = mp.tile([128, NG], f32, tag="otp", name=f"otp{m}")
                for k in range(2):
                    nc.tensor.matmul(
                        otp[:],
                        w2_sb[:, k * H + m * 128 : k * H + m * 128 + 128],
                        h1t[k][:],
                        start=(k == 0),
                        stop=(k == 1),
                    )
                osb = ms.tile([128, NG], f32, tag="osb", name=f"osb{m}")
                nc.vector.tensor_scalar_add(osb[:], otp[:], b2_sb[:, m : m + 1])
                out_eng = nc.sync if m == 0 else nc.scalar
                out_eng.dma_start(outT_d[m * 128 : (m + 1) * 128, :], osb[:])

    return nc


def get_program(slot_tiles: tuple[int, ...]) -> "bass.Bass":
    if slot_tiles not in _PROGRAM_CACHE:
        nc = build_program(slot_tiles)
        # HW-path only (CoreSim snapshots the program before this pass)
        _fix_excess_waits(nc)
        _PROGRAM_CACHE[slot_tiles] = nc
    return _PROGRAM_CACHE[slot_tiles]


# ---------------------------------------------------------------------------
# Host-side sharding / padding


def plan_shards(batch: np.ndarray):
    """Returns (assign [NCORES][NG] graph ids, slot_tiles tuple, sizes)."""
    sizes = np.bincount(batch, minlength=B).astype(np.int64)
    order = np.argsort(-sizes, kind="stable")
    assign = [[] for _ in range(NCORES)]
    for r in range(NG):
        row = order[r * NCORES : (r + 1) * NCORES]
        if r % 2 == 1:
            row = row[::-1]
        for c in range(NCORES):
            assign[c].append(int(row[c]))
    for c in range(NCORES):
        assign[c].sort(key=lambda g: -sizes[g])
    slot_tiles = []
    for j in range(NG):
        mx = max(sizes[assign[c][j]] for c in range(NCORES))
        slot_tiles.append(int(max(1, -(-mx // TILE))))
    # round total tiles up to a GROUP multiple (pad goes to the last slot)
    rem = (-sum(slot_tiles)) % (GROUP // TILE)
    slot_tiles[-1] += rem
    return assign, tuple(slot_tiles), sizes


def make_in_maps(edge_features, batch, seed_vectors, w_q, w_k, w_v, w1, b1, w2, b2):
    edge_features = np.asarray(edge_features, dtype=np.float32)
    batch = np.asarray(batch)
    assign, slot_tiles, sizes = plan_shards(batch)
    TT = sum(slot_tiles)
    EC = TT * TILE

    starts = np.searchsorted(batch, np.arange(B))
    xb = edge_features.astype(BF16)

    # Ws[hin, h*S+s] = sum_d w_k[hin, h*HD+d] * q[s, h, d] / sqrt(HD)
    q = (np.asarray(seed_vectors, np.float32) @ np.asarray(w_q, np.float32)).reshape(
        S, NH, HD
    )
    wk3 = np.asarray(w_k, np.float32).reshape(H, NH, HD)
    Ws = (np.einsum("ihd,shd->ihs", wk3, q) * SCALE).reshape(H, NH * S)
    wvs = np.concatenate([np.asarray(w_v, np.float32), Ws], axis=1)

    # w1 permuted so k-block j = (half m = j//32, seed s = j%32) matches the
    # per-graph-contiguous P2 layout; 0.5x on w2 folds silu's tanh constant.
    w1p = (
        np.asarray(w1, np.float32)
        .reshape(S, 2, 128, H)
        .transpose(1, 0, 2, 3)
        .reshape(S * H, H)
    )
    shared = {
        "wvs": np.ascontiguousarray(wvs.astype(BF16)),
        "w1": np.ascontiguousarray(w1p.astype(BF16)),
        "w2": np.ascontiguousarray((0.5 * np.asarray(w2, np.float32)).astype(BF16)),
        "b1": np.ascontiguousarray(
            np.broadcast_to(np.asarray(b1, dtype=np.float32), (NG, H))
        ),
        "b2": np.ascontiguousarray(np.asarray(b2, dtype=np.float32).reshape(H, 1)),
        "ident": np.eye(128, dtype=BF16),
    }

    in_maps = []
    for c in range(NCORES):
        xt = np.zeros((H, EC), dtype=BF16)
        npad = np.zeros(NG, dtype=np.float32)
        off = 0
        for j, g in enumerate(assign[c]):
            n = int(sizes[g])
            xt[:, off : off + n] = xb[starts[g] : starts[g] + n].T
            npad[j] = slot_tiles[j] * TILE - n
            off += slot_tiles[j] * TILE
        m = dict(shared)
        m["xt"] = xt
        m["npad"] = np.ascontiguousarray(np.broadcast_to(npad, (128, NG)))
        in_maps.append(m)
    return in_maps, assign, slot_tiles


def kernel(
    edge_features,
    edge_coords,
    batch,
    seed_vectors,
    w_q,
    w_k,
    w_v,
    w1,
    b1,
    w2,
    b2,
):
    in_maps, assign, slot_tiles = make_in_maps(
        edge_features, batch, seed_vectors, w_q, w_k, w_v, w1, b1, w2, b2
    )
    nc = get_program(slot_tiles)

    res = run_bass_kernel_spmd(nc, in_maps, core_ids=list(range(NCORES)))
    global LAST_RESULTS
    LAST_RESULTS = res

    out = np.zeros((B, H), dtype=np.float32)
    for c in range(NCORES):
        outT = res.results[c]["outT"]  # [H, NG]
        for j, g in enumerate(assign[c]):
            out[g, :] = outT[:, j]
    return out



# revision 40
# speedup vs baseline: 1.6395x; 1.0209x over previous
"""AttentionPooling (ragged graph cross-attention pooling) on 8 TRN2 NeuronCores.

Strategy (SPMD, no collectives):
  * Host assigns 8 whole graphs to each of the 8 cores (serpentine by size),
    sorts each core's graphs by size into 8 "slots".  Slot j has a fixed tile
    count T[j] (shared by all cores, since the instruction stream is shared);
    each graph's edges are placed at its slot offset and zero-padded.
  * Host ships x^T (transposed edge features, bf16) per core + replicated
    weights.  Padding edges give exp(0)=1 in the softmax denominator, which is
    corrected with a host-computed per-slot pad count.
  * Softmax is computed without max-subtraction (scores ~ N(0,1); exp cannot
    overflow fp32) — mathematically identical to the reference's stable form.
  * Scores are linear in x: scores = (x @ w_k) . q  =  x @ Ws where
    Ws[:, (h,s)] = sum_d w_k[:, (h,d)] q[s,h,d] / sqrt(hd).  Ws ([256, 256])
    is host-precomputed from the weights and shipped fused with w_v as one
    [256, 512] operand, so the per-tile device work is:
      [v | sc][e, :] = x @ [w_v | Ws]    (PE, 2 matmuls/tile, N=512)
      ex             = exp(sc)           (ACT, psum->sbuf bf16)
      pooled[(h,s),(h,d)|denom] += ex.T @ [v | 1]  (PE, psum-accum per graph)
  * Per graph: denom -= npad; recip (DVE); normalize out of PSUM into bf16
    (ACT Copy w/ scale for m=0, DVE for m=1); 32x32 block transpose (DVE
    StreamTranspose); packed 32-col copies into P2, laid out graph-major
    [128, g*64 + (half*32+seed)] so extract writes are contiguous and the
    MLP reads graph-strided [128, 8] blocks via the LDWEIGHTS pattern (w1
    is host-permuted to matching (half, seed)-major k-blocks).
  * MLP: h1pp = P2^T @ w1 (PE, 4-way tile_position col-group packing, the
    4 quadrant strips summed on DVE with b1), silu via x*0.5*(1+tanh(x/2))
    (tanh shares the exp ACT table - no mid-kernel table reload; the 0.5 is
    folded into w2 host-side), out = h1 @ w2' + b2 (PE) emitted as out^T
    [256, 8] per core; the host scatters core outputs into [64, 256].
  * Schedule notes: exp table warms before anything else on ACT; const DMAs
    ride gpsimd; tail-only consts + w1 (in 8 thin slices) are pinned behind
    mid-loop ex tiles via 1-elem dummy copies so the Tile scheduler cannot
    hoist their transfers into the xt ramp-up; xt_pool bufs=10 covers the
    DMA bandwidth-delay product; dependency-free filler matmuls bridge the
    loop-drain and extract gaps so the PE p-state clock stays at full rate
    into the MLP; the Tile teardown skips the per-semaphore clear chain
    (one barrier only) since the NEFF runs once per launch.
"""

import os
import sys
from contextlib import ExitStack

import numpy as np

for _p in ("/opt/trn_rl_repo",):
    if _p not in sys.path:
        sys.path.append(_p)

import ml_dtypes  # noqa: E402

import concourse.bass as bass  # noqa: E402
import concourse.tile as tile  # noqa: E402
from concourse import mybir  # noqa: E402
from concourse.bass_utils import run_bass_kernel_spmd  # noqa: E402
from concourse.vector_clock import ScopedClock  # noqa: E402

BF16 = ml_dtypes.bfloat16

E, B, H, S, NH, HD = 131072, 64, 256, 32, 8, 32
NCORES = 8
NG = B // NCORES        # graphs (slots) per core
TILE = 128              # edge tile
GROUP = 512             # x^T DMA chunk (4 tiles)
SCALE = 1.0 / float(np.sqrt(HD))

AF = mybir.ActivationFunctionType

# ---------------------------------------------------------------------------
# Walrus workaround: this toolchain's InstDrain accepts only ONE sync wait;
# Tile's kernel-tail drain carries one wait per outstanding semaphore.
# Split it into a chain of single-wait drains.
_MAXW = 1


def _split_drain_and_barrier(self, tick_clock, wait_clock):
    nc = self.nc
    drain_inst = nc.sync.drain()
    wait_clock.add_sem_waits(
        drain_inst.ins, ScopedClock({None: tick_clock.global_clock})
    )
    waits = list(drain_inst.ins.sync_info.on_wait)
    if len(waits) > _MAXW:
        drain_inst.ins.sync_info = mybir.SyncInfo(on_wait=waits[:_MAXW], on_update=[])
        for i in range(_MAXW, len(waits), _MAXW):
            d2 = nc.sync.drain()
            d2.ins.sync_info = mybir.SyncInfo(
                on_wait=waits[i : i + _MAXW], on_update=[]
            )
    popped = nc._tile_sem_poison_stack.pop()
    assert popped is self._sem_poison
    # Program runs once per NEFF execution; skip the per-semaphore clear
    # chain (several us of EVENT_SEMAPHORE resets) and settle for one
    # barrier.  Free the handles bookkeeping-only so later allocations
    # (none today) stay consistent.
    nc._state.prepend_free_semaphores(
        [getattr(s, "num", s) for s in self.sems.allocated().values()]
    )
    nc.all_engine_barrier()


tile.TileContext._drain_and_barrier = _split_drain_and_barrier

# Engine instructions are capped at 2 sync waits by this walrus (Drain/NoOp
# at 1).  Tile's sem-assignment occasionally emits more.  Hoist the excess
# onto single-wait NoOps inserted just before, on the same engine — the
# engine stalls at the NoOp instead, which is semantically identical.
_WAIT_CAP = {"InstDrain": 1}
_WAIT_CAP_DEFAULT = 1


def _fix_excess_waits(nc):
    n_fixed = 0
    for fn in nc.m.functions:
        for bb in fn.blocks:
            insts = bb.instructions
            out = []
            changed = False
            for inst in insts:
                si = inst.sync_info
                waits = list(si.on_wait) if si is not None else []
                cap = _WAIT_CAP.get(type(inst).__name__, _WAIT_CAP_DEFAULT)
                if len(waits) > cap:
                    changed = True
                    n_fixed += 1
                    excess = waits[: len(waits) - cap]
                    for i, w in enumerate(excess):
                        nop = mybir.InstNoOp(
                            name=f"{inst.name}-hw{i}", ins=[], outs=[]
                        )
                        nop.engine = inst.engine
                        nop.sync_info = mybir.SyncInfo(on_wait=[w], on_update=[])
                        out.append(nop)
                    inst.sync_info = mybir.SyncInfo(
                        on_wait=waits[len(excess) :], on_update=list(si.on_update)
                    )
                out.append(inst)
            if changed:
                bb.instructions = out
    return n_fixed

# ---------------------------------------------------------------------------

_PROGRAM_CACHE: dict[tuple, "bass.Bass"] = {}
LAST_RESULTS = None  # BassKernelResults of the most recent run (for testing)


def _install_ntff_hook_shim():
    """The image's antenv lacks axon_hooks; recreate it so trace=True works."""
    try:
        import types

        import antenv

        if "antenv.axon_hooks" not in sys.modules:
            mod = types.ModuleType("antenv.axon_hooks")
            mod._hook = None

            def set_axon_ntff_profile_hook(h):
                mod._hook = h

            def get_axon_ntff_profile_hook():
                return mod._hook

            mod.set_axon_ntff_profile_hook = set_axon_ntff_profile_hook
            mod.get_axon_ntff_profile_hook = get_axon_ntff_profile_hook
            sys.modules["antenv.axon_hooks"] = mod
            antenv.axon_hooks = mod
        import antenv.axon_hooks as ah

        if ah.get_axon_ntff_profile_hook() is None:
            from trn_agent_boot.trn_boot import _ntff_profile_via_ctypes

            ah.set_axon_ntff_profile_hook(
                _ntff_profile_via_ctypes("/opt/axon/libaxon_pjrt.so")
            )
    except Exception:
        pass


_install_ntff_hook_shim()

# Optional experiment: let walrus double-buffer LDWEIGHTS (default off here).
import concourse.bass_utils as _bass_utils  # noqa: E402

_orig_run_command = _bass_utils.run_command


def _run_command_ldwopt(cmd, **kw):
    if isinstance(cmd, list):
        cmd = [
            "--enable-ldw-opt=true" if c == "--enable-ldw-opt=false" else c
            for c in cmd
        ]
    return _orig_run_command(cmd, **kw)


if os.environ.get("KERNEL_LDW_OPT") == "1":
    _bass_utils.run_command = _run_command_ldwopt


def build_program(slot_tiles: tuple[int, ...]) -> "bass.Bass":
    """Build the SPMD Bass program for per-core slot tile counts."""
    TT = sum(slot_tiles)
    EC = TT * TILE
    assert TT % (GROUP // TILE) == 0
    NGRP = TT // (GROUP // TILE)

    # per-tile slot id / first / last flags
    slot_of, first_of, last_of = [], [], []
    for j, tj in enumerate(slot_tiles):
        for t in range(tj):
            slot_of.append(j)
            first_of.append(t == 0)
            last_of.append(t == tj - 1)

    f32, bf16 = mybir.dt.float32, mybir.dt.bfloat16
    nc = bass.Bass("TRN2", target_bir_lowering=False, debug=False, num_devices=NCORES)

    xt_d = nc.dram_tensor("xt", [H, EC], bf16, kind="ExternalInput").ap()
    wvs_d = nc.dram_tensor("wvs", [H, 2 * H], bf16, kind="ExternalInput").ap()
    w1_d = nc.dram_tensor("w1", [S * H, H], bf16, kind="ExternalInput").ap()
    w2_d = nc.dram_tensor("w2", [H, H], bf16, kind="ExternalInput").ap()
    b1_d = nc.dram_tensor("b1", [NG, H], f32, kind="ExternalInput").ap()
    b2_d = nc.dram_tensor("b2", [H, 1], f32, kind="ExternalInput").ap()
    npad_d = nc.dram_tensor("npad", [128, NG], f32, kind="ExternalInput").ap()
    ident_d = nc.dram_tensor("ident", [128, 128], bf16, kind="ExternalInput").ap()
    outT_d = nc.dram_tensor("outT", [H, NG], f32, kind="ExternalOutput").ap()

    with tile.TileContext(nc) as tc, ExitStack() as ctx:
        const = ctx.enter_context(tc.tile_pool(name="const", bufs=1))
        w2_sb = const.tile([128, 2 * H], bf16)
        wvs_sb = const.tile([128, 2 * 2 * H], bf16)  # k-tile k: [wv_k | ws_k]
        w1_sb = const.tile([128, 64 * H], bf16)
        ident_sb = const.tile([128, 128], bf16)
        b1_sb = const.tile([NG, H], f32)
        b2_sb = const.tile([128, 2], f32)
        npad_sb = const.tile([128, NG], f32)
        P2 = const.tile([128, 64 * NG], bf16)

        # Warm the exp ACT table FIRST: the scalar engine must not issue
        # anything before this, so the (1.5us) table load overlaps the
        # startup DMAs instead of trailing them.  The whole kernel uses
        # only table-0 functions (Exp/Copy/Identity) — no reloads ever.
        warm = const.tile([1, 2], f32)
        nc.vector.memset(warm[:, 0:1], 0.0)
        nc.scalar.activation(warm[:, 1:2], warm[:, 0:1], AF.Exp)

        # Const DMAs ride the gpsimd queue: scalar stays free for exp,
        # sync stays free for the xt stream.  wvs goes first — the first
        # vsc matmul blocks on it.
        for k in range(2):
            r = slice(k * 128, (k + 1) * 128)
            nc.gpsimd.dma_start(wvs_sb[:, k * 2 * H : (k + 1) * 2 * H], wvs_d[r, :])
        nc.gpsimd.dma_start(npad_sb[:], npad_d[:])

        # ---- main edge loop ---------------------------------------------
        xt_pool = ctx.enter_context(tc.tile_pool(name="xtp", bufs=10))
        ex_pool = ctx.enter_context(tc.tile_pool(name="exp", bufs=10))
        ext_pool = ctx.enter_context(tc.tile_pool(name="ext", bufs=3))

        NRING = 10
        vs_ring = [const.tile([128, 258], bf16, name=f"vsring{i}") for i in range(NRING)]
        for t in vs_ring:
            nc.vector.memset(t[:, 128:129], 1.0)
            nc.vector.memset(t[:, 257:258], 1.0)

        pooled_tiles: list = [None, None]

        def emit_pooled(sl, fi, la, ex, vs):
            if fi:
                pooled_tiles[0] = pl_pool.tile([128, 129], f32, tag="pl0", name=f"pl0_s{sl}")
                pooled_tiles[1] = pl_pool.tile([128, 129], f32, tag="pl1", name=f"pl1_s{sl}")
            for m in range(2):
                nc.tensor.matmul(
                    pooled_tiles[m][:],
                    ex[:, m * 128 : (m + 1) * 128],
                    vs[:, m * 129 : m * 129 + 129],
                    start=fi,
                    stop=la,
                )
            if la:
                extract_graph(sl, pooled_tiles)

        # P2 layout: column g*64 + m*32 + s  (graph-major, then (half, seed)).
        # Extract writes CONTIGUOUS 32-col runs (fast DVE/Pool path); the MLP
        # reads graph-strided [128, NG] blocks via the LDWEIGHTS pattern.
        P2v = P2[:].rearrange("p (g j) -> p g j", j=64)

        def extract_graph(g, ptiles):
            copy_eng = nc.vector if g == NG - 1 else nc.gpsimd
            for m in range(2):
                den = ext_pool.tile([128, 1], f32, tag="den", name=f"den{g}_{m}")
                nc.vector.tensor_scalar_sub(
                    den[:], ptiles[m][:, 128:129], npad_sb[:, g : g + 1]
                )
                rec = ext_pool.tile([128, 1], f32, tag="rec", name=f"rec{g}_{m}")
                nc.vector.reciprocal(rec[:], den[:])
                # normalize straight out of PSUM into bf16, split across the
                # ACT engine (m=0) and DVE (m=1) so neither engine eats the
                # whole slot-boundary burst.
                pn = ext_pool.tile([128, 128], bf16, tag="pn", name=f"pn{g}_{m}")
                if m == 0:
                    nc.scalar.activation(
                        pn[:], ptiles[m][:, 0:128], AF.Copy, scale=rec[:]
                    )
                else:
                    nc.vector.tensor_scalar_mul(pn[:], ptiles[m][:, 0:128], rec[:])
                pt = ext_pool.tile([128, 128], bf16, tag="pt", name=f"pt{g}_{m}")
                nc.vector.transpose(pt[:], pn[:])
                for hh in range(4):
                    rr = slice(hh * 32, (hh + 1) * 32)
                    copy_eng.tensor_copy(
                        P2v[rr, g, m * 32 : (m + 1) * 32],
                        pt[rr, hh * 32 : (hh + 1) * 32],
                    )

        with (
            tc.tile_pool(name="vscp", bufs=4, space="PSUM") as vsc_pool,
            tc.tile_pool(name="plp", bufs=2, space="PSUM") as pl_pool,
        ):
            from collections import deque

            pending = deque()
            tidx = 0
            for grp in range(NGRP):
                xt = [
                    xt_pool.tile([128, GROUP], bf16, tag="xt", name=f"xt_{grp}_{i}")
                    for i in range(2)
                ]
                for k in range(2):
                    nc.sync.dma_start(
                        xt[k][:],
                        xt_d[k * 128 : (k + 1) * 128, grp * GROUP : (grp + 1) * GROUP],
                    )
                for sub in range(4):
                    sl, fi, la = slot_of[tidx], first_of[tidx], last_of[tidx]
                    e0 = sub * TILE
                    vsc = vsc_pool.tile([128, 512], f32, tag="vsc", name=f"vsc{tidx}")
                    for k in range(2):
                        nc.tensor.matmul(
                            vsc[:],
                            xt[k][:, e0 : e0 + TILE],
                            wvs_sb[:, k * 2 * H : (k + 1) * 2 * H],
                            start=(k == 0),
                            stop=(k == 1),
                        )
                    ex = ex_pool.tile([128, 256], bf16, tag="ex", name=f"ex{tidx}")
                    nc.scalar.activation(ex[:], vsc[:, H : 2 * H], AF.Exp)
                    vs = vs_ring[tidx % NRING]
                    nc.vector.tensor_copy(
                        vs[:].rearrange("p (b c) -> p b c", c=129)[:, :, 0:128],
                        vsc[:, 0:H].rearrange("p (b c) -> p b c", c=128),
                    )
                    if tidx == 24:
                        # Tail-only consts must NOT transfer during the xt
                        # ramp-up.  The Tile scheduler hoists dependency-free
                        # DMAs to the stream front, so pin each behind this
                        # tile's ex via a 1-element dummy copy (the DMA then
                        # overwrites it with real data).
                        for tgt in (w2_sb, ident_sb):
                            nc.gpsimd.tensor_copy(tgt[0:1, 0:1], ex[0:1, 0:1])
                        for k in range(2):
                            r = slice(k * 128, (k + 1) * 128)
                            nc.gpsimd.dma_start(
                                w2_sb[:, k * H : (k + 1) * H], w2_d[r, :]
                            )
                            nc.gpsimd.dma_start(b2_sb[:, k : k + 1], b2_d[r, :])
                        nc.gpsimd.dma_start(ident_sb[:], ident_d[:])
                        nc.gpsimd.dma_start(b1_sb[:], b1_d[:])
                    if tidx >= 28 and tidx % 12 == 4 and (si := (tidx - 28) // 12) < 8:
                        # w1 (4MB) in 8 thin slices spread across the loop:
                        # ~512KB every ~12 tiles keeps its HBM draw far below
                        # the xt stream's needs.
                        nc.gpsimd.tensor_copy(
                            w1_sb[0:1, si * 8 * H : si * 8 * H + 1], ex[0:1, 0:1]
                        )
                        nc.gpsimd.dma_start(
                            w1_sb[:, si * 8 * H : (si + 1) * 8 * H].rearrange(
                                "p (k c) -> p k c", c=H
                            ),
                            w1_d[si * 1024 : (si + 1) * 1024, :].rearrange(
                                "(k p) c -> p k c", p=128
                            ),
                        )
                    pending.append((sl, fi, la, ex, vs))
                    while len(pending) > 5:
                        emit_pooled(*pending.popleft())
                    tidx += 1
            # Drain phase: pooled emissions now run at copy-latency rhythm
            # with 300-500ns PE gaps that would reset the PE p-state ramp
            # (halving the MLP's clock).  Interleave dependency-free filler
            # matmuls to keep the clock hot into the tail.
            scratch = vsc_pool.tile([128, 512], f32, tag="vsc", name="scratch")
            while pending:
                emit_pooled(*pending.popleft())
                for _ in range(2):
                    nc.tensor.matmul(
                        scratch[:, 0:256], ident_sb[:], wvs_sb[:, 0:256],
                        start=True, stop=True,
                    )

        # ---- MLP tail ----------------------------------------------------
        with (
            tc.tile_pool(name="mlpp", bufs=2, space="PSUM") as mp,
            tc.tile_pool(name="mlps", bufs=2) as ms,
        ):
            # Keep the PE p-state hot across the last-graph extract: a few
            # dependency-free filler matmuls bridge the idle gap so h1pp
            # streams at full clock instead of the half-rate ramp.
            warmmm = mp.tile([128, 256], f32, tag="warmmm")
            for wi in range(6):
                nc.tensor.matmul(
                    warmmm[:], ident_sb[:], wvs_sb[:, 0:256],
                    start=True, stop=True,
                )
            h1pp = mp.tile([128, H], f32, tag="h1pp")
            for j in range(64):
                q = j % 4
                nc.tensor.matmul(
                    h1pp[q * 32 : q * 32 + NG, :],
                    P2v[:, :, j],
                    w1_sb[:, j * H : (j + 1) * H],
                    start=(j < 4),
                    stop=(j >= 60),
                    tile_position=(0, q * 32),
                    skip_group_check=True,
                )
            # quadrant sum + b1 on DVE/Pool straight from PSUM (no qsel
            # matmul); only one PSUM operand per instruction is legal.
            # pure-vector accumulation chain: Tile's wait coalescing
            # serializes cross-engine PSUM readers anyway, so avoid the
            # extra engine hops entirely.
            t01 = ms.tile([NG, H], f32, tag="t01")
            nc.vector.tensor_add(t01[:], h1pp[0:NG, :], b1_sb[:])
            t02 = ms.tile([NG, H], f32, tag="t02")
            nc.vector.tensor_add(t02[:], h1pp[32 : 32 + NG, :], t01[:])
            t03 = ms.tile([NG, H], f32, tag="t03")
            nc.vector.tensor_add(t03[:], h1pp[64 : 64 + NG, :], t02[:])
            h1s = ms.tile([NG, H], f32, tag="h1s")
            nc.vector.tensor_add(h1s[:], h1pp[96 : 96 + NG, :], t03[:])
            # silu(x) = x * sigmoid(x) = 0.5*x*(1 + tanh(x/2)); tanh lives in
            # the exp ACT table (no reload) and the 0.5 is folded into w2
            # host-side.
            h1e = ms.tile([NG, H], f32, tag="h1e")
            nc.scalar.activation(h1e[:], h1s[:], AF.Tanh, scale=0.5)
            h1d = ms.tile([NG, H], f32, tag="h1d")
            nc.vector.tensor_scalar_add(h1d[:], h1e[:], 1.0)
            h1b = ms.tile([NG, H], bf16, tag="h1b")
            nc.vector.tensor_mul(h1b[:], h1s[:], h1d[:])
            h1t = []
            for m in range(2):
                h1tp = mp.tile([128, NG], bf16, tag="h1tp", name=f"h1tp{m}")
                nc.tensor.transpose(
                    h1tp[:], h1b[:, m * 128 : (m + 1) * 128], ident_sb[0:NG, 0:NG]
                )
                ht = ms.tile([128, NG], bf16, tag=f"h1t{m}")
                nc.vector.tensor_copy(ht[:], h1tp[:])
                h1t.append(ht)
            for m in range(2):
                otp = mp.tile([128, NG], f32, tag="otp", name=f"otp{m}")
                for k in range(2):
                    nc.tensor.matmul(
                        otp[:],
                        w2_sb[:, k * H + m * 128 : k * H + m * 128 + 128],
                        h1t[k][:],
                        start=(k == 0),
                        stop=(k == 1),
                    )
                osb = ms.tile([128, NG], f32, tag="osb", name=f"osb{m}")
                nc.vector.tensor_scalar_add(osb[:], otp[:], b2_sb[:, m : m + 1])
                out_eng = nc.sync if m == 0 else nc.scalar
                out_eng.dma_start(outT_d[m * 128 : (m + 1) * 128, :], osb[:])

    return nc


def get_program(slot_tiles: tuple[int, ...]) -> "bass.Bass":
    if slot_tiles not in _PROGRAM_CACHE:
        nc = build_program(slot_tiles)
        # HW-path only (CoreSim snapshots the program before this pass)
        _fix_excess_waits(nc)
        _PROGRAM_CACHE[slot_tiles] = nc
    return _PROGRAM_CACHE[slot_tiles]


# ---------------------------------------------------------------------------
# Host-side sharding / padding


def plan_shards(batch: np.ndarray):
    """Returns (assign [NCORES][NG] graph ids, slot_tiles tuple, sizes)."""
    sizes = np.bincount(batch, minlength=B).astype(np.int64)
    order = np.argsort(-sizes, kind="stable")
    assign = [[] for _ in range(NCORES)]
    for r in range(NG):
        row = order[r * NCORES : (r + 1) * NCORES]
        if r % 2 == 1:
            row = row[::-1]
        for c in range(NCORES):
            assign[c].append(int(row[c]))
    for c in range(NCORES):
        assign[c].sort(key=lambda g: -sizes[g])
    slot_tiles = []
    for j in range(NG):
        mx = max(sizes[assign[c][j]] for c in range(NCORES))
        slot_tiles.append(int(max(1, -(-mx // TILE))))
    # round total tiles up to a GROUP multiple (pad goes to the last slot)
    rem = (-sum(slot_tiles)) % (GROUP // TILE)
    slot_tiles[-1] += rem
    return assign, tuple(slot_tiles), sizes


def make_in_maps(edge_features, batch, seed_vectors, w_q, w_k, w_v, w1, b1, w2, b2):
    edge_features = np.asarray(edge_features, dtype=np.float32)
    batch = np.asarray(batch)
    assign, slot_tiles, sizes = plan_shards(batch)
    TT = sum(slot_tiles)
    EC = TT * TILE

    starts = np.searchsorted(batch, np.arange(B))
    xb = edge_features.astype(BF16)

    # Ws[hin, h*S+s] = sum_d w_k[hin, h*HD+d] * q[s, h, d] / sqrt(HD)
    q = (np.asarray(seed_vectors, np.float32) @ np.asarray(w_q, np.float32)).reshape(
        S, NH, HD
    )
    wk3 = np.asarray(w_k, np.float32).reshape(H, NH, HD)
    Ws = (np.einsum("ihd,shd->ihs", wk3, q) * SCALE).reshape(H, NH * S)
    wvs = np.concatenate([np.asarray(w_v, np.float32), Ws], axis=1)

    # w1 permuted so k-block j = (half m = j//32, seed s = j%32) matches the
    # per-graph-contiguous P2 layout; 0.5x on w2 folds silu's tanh constant.
    w1p = (
        np.asarray(w1, np.float32)
        .reshape(S, 2, 128, H)
        .transpose(1, 0, 2, 3)
        .reshape(S * H, H)
    )
    shared = {
        "wvs": np.ascontiguousarray(wvs.astype(BF16)),
        "w1": np.ascontiguousarray(w1p.astype(BF16)),
        "w2": np.ascontiguousarray((0.5 * np.asarray(w2, np.float32)).astype(BF16)),
        "b1": np.ascontiguousarray(
            np.broadcast_to(np.asarray(b1, dtype=np.float32), (NG, H))
        ),
        "b2": np.ascontiguousarray(np.asarray(b2, dtype=np.float32).reshape(H, 1)),
        "ident": np.eye(128, dtype=BF16),
    }

    in_maps = []
    for c in range(NCORES):
        xt = np.zeros((H, EC), dtype=BF16)
        npad = np.zeros(NG, dtype=np.float32)
        off = 0
        for j, g in enumerate(assign[c]):
            n = int(sizes[g])
            xt[:, off : off + n] = xb[starts[g] : starts[g] + n].T
            npad[j] = slot_tiles[j] * TILE - n
            off += slot_tiles[j] * TILE
        m = dict(shared)
        m["xt"] = xt
        m["npad"] = np.ascontiguousarray(np.broadcast_to(npad, (128, NG)))
        in_maps.append(m)
    return in_maps, assign, slot_tiles


def kernel(
    edge_features,
    edge_coords,
    batch,
    seed_vectors,
    w_q,
    w_k,
    w_v,
    w1,
    b1,
    w2,
    b2,
):
    in_maps, assign, slot_tiles = make_in_maps(
        edge_features, batch, seed_vectors, w_q, w_k, w_v, w1, b1, w2, b2
    )
    nc = get_program(slot_tiles)

    res = run_bass_kernel_spmd(nc, in_maps, core_ids=list(range(NCORES)))
    global LAST_RESULTS
    LAST_RESULTS = res

    out = np.zeros((B, H), dtype=np.float32)
    for c in range(NCORES):
        outT = res.results[c]["outT"]  # [H, NG]
        for j, g in enumerate(assign[c]):
            out[g, :] = outT[:, j]
    return out

